# revision 1
# baseline (speedup 1.0000x reference)
"""Trainium2 Bass kernel for nn_FAEncoder: frame-averaged SRU++ encoder.

Sharding: data-parallel over B (8 samples -> 8 cores). Each core processes its
sample's 8 sign-flip frames end to end; weights are replicated. No collectives.

Device algorithm notes:
- Activations are feature-major (feature on partitions, token on free),
  token index = frame*512 + l.
- Softmax runs in the (m-on-partitions) layout without max subtraction (scores
  are bounded ~0.1 here); the denominator comes from a ones-vector matmul and
  is applied via a K=1 broadcast matmul.
- The SRU recurrence c_t = f_t*c_{t-1} + (1-f_t)*u0_t (f_t depends on c_{t-1})
  is solved by fixed-point iteration: freezing gates at the previous iterate
  makes the scan linear, mapping onto DVE tensor_tensor_scan. Contraction is
  ~0.02/iteration (|vf| ~ 0.1), so it0 + NMID mids + final reaches near-fp32
  accuracy on this data distribution.
- The 3x3 eigh runs on device: cyclic Jacobi + ascending sort network.
- Matmuls use float32r (~1 cycle/row at N=512, ~2e-4 operand rounding). The
  walrus verifier requires every producer of an fp32r matmul operand to write
  fp32r, so matmul-feeding tiles are declared float32r and non-matmul readers
  bitcast them back to float32.
"""
import os
import sys
from contextlib import ExitStack

import numpy as np

for _p in ('/opt/trn_rl_repo', '/root/.axon_site/_ro/trn_rl_repo'):
    if os.path.isdir(_p) and _p not in sys.path:
        sys.path.append(_p)

import concourse.bacc as bacc
import concourse.bass as bass
import concourse.mybir as mybir
from concourse.bass_utils import run_bass_kernel_spmd
from concourse.masks import make_identity
from concourse.tile import TileContext

F32 = mybir.dt.float32
R32 = mybir.dt.float32r
I32 = mybir.dt.int32
AF = mybir.ActivationFunctionType
OP = mybir.AluOpType

B, N, H, D2, EMB, VOCAB = 8, 512, 256, 128, 256, 100
NF, ND = 8, 2
P = 128
SCALE = float(1.0 / np.sqrt(np.float32(D2)))
NMID = 1      # fixed-point mid iterations between it0 and the final pass
SWEEPS = 3    # cyclic Jacobi sweeps for the 3x3 eigh


def _build(n_iter=1):
    nc = bacc.Bacc('TRN2', num_devices=B, debug=False)

    def mm(out, lhsT, rhs, start, stop):
        nc.tensor.matmul(out, lhsT, rhs, start=start, stop=stop)

    def f32(ap):
        return ap.bitcast(F32)

    dt = nc.dram_tensor
    x_ca = dt('x_ca', [N, 3], F32, kind='ExternalInput').ap()
    maskr = dt('maskr', [1, N], F32, kind='ExternalInput').ap()
    seq = dt('seq', [4, P], I32, kind='ExternalInput').ap()
    emb_d = dt('emb', [VOCAB, EMB], F32, kind='ExternalInput').ap()
    sops = dt('sops', [3, 24], F32, kind='ExternalInput').ap()
    wq = [dt('wq0', [3 + EMB, D2], R32, kind='ExternalInput').ap(),
          dt('wq1', [H, D2], R32, kind='ExternalInput').ap()]
    wk = [dt(f'wk{i}', [D2, D2], R32, kind='ExternalInput').ap() for i in range(2)]
    wv = [dt(f'wv{i}', [D2, D2], R32, kind='ExternalInput').ap() for i in range(2)]
    wu = [dt(f'wu{i}', [D2, 8 * D2], R32, kind='ExternalInput').ap() for i in range(2)]
    gates = [dt(f'gates{i}', [8, P], F32, kind='ExternalInput').ap() for i in range(2)]
    out_d = dt('out', [N, H], F32, kind='ExternalOutput').ap()

    jac_scr = dt('jac_scr', [16], F32, kind='Internal').ap()
    hx_scr = dt('hx_scr', [NF, 3, N], R32, kind='Internal').ap()

    with TileContext(nc) as tc, ExitStack() as es:
        cst = es.enter_context(tc.tile_pool(name='cst', bufs=1))
        pre = es.enter_context(tc.tile_pool(name='pre', bufs=2))
        att = es.enter_context(tc.tile_pool(name='att', bufs=2))
        ust = es.enter_context(tc.tile_pool(name='ust', bufs=2))
        scn = es.enter_context(tc.tile_pool(name='scn', bufs=3))
        sc2 = es.enter_context(tc.tile_pool(name='sc2', bufs=2))
        h2p = es.enter_context(tc.tile_pool(name='h2p', bufs=1))
        jcb = es.enter_context(tc.tile_pool(name='jcb', bufs=1))
        pmm = es.enter_context(tc.tile_pool(name='pmm', bufs=2, space='PSUM'))
        psc = es.enter_context(tc.tile_pool(name='psc', bufs=2, space='PSUM'))
        pzt = es.enter_context(tc.tile_pool(name='pzt', bufs=2, space='PSUM'))
        pau = es.enter_context(tc.tile_pool(name='pau', bufs=2, space='PSUM'))

        def body():
            # ---------------- constants / weight staging ----------------
            ident = cst.tile([P, P], F32, tag='ident', name='ident')
            make_identity(nc, ident)
            ident_r = cst.tile([P, P], R32, tag='ident_r', name='ident_r')
            nc.scalar.copy(ident_r[:], ident[:])
            ones_col = cst.tile([P, 1], F32, tag='ones_col', name='ones_col')
            nc.vector.memset(ones_col[:], 1.0)
            ones_col_r = cst.tile([P, 1], R32, tag='ones_col_r', name='ones_col_r')
            nc.scalar.copy(ones_col_r[:], ones_col[:])
            ones_row = cst.tile([1, P], F32, tag='ones_row', name='ones_row')
            nc.vector.memset(ones_row[:], 1.0)
            ones_row_r = cst.tile([1, P], R32, tag='ones_row_r', name='ones_row_r')
            nc.scalar.copy(ones_row_r[:], ones_row[:])
            zcol = cst.tile([P, 1], F32, tag='zcol', name='zcol')
            nc.vector.memset(zcol[:], 0.0)

            s_sb = cst.tile([3, 24], F32, tag='s_sb', name='s_sb')
            nc.gpsimd.dma_start(s_sb[:], sops)

            wq_sb = [[cst.tile([3, D2], R32, tag='wq0a', name='wq0a'),
                      cst.tile([P, D2], R32, tag='wq0b', name='wq0b'),
                      cst.tile([P, D2], R32, tag='wq0c', name='wq0c')],
                     [cst.tile([P, D2], R32, tag='wq1a', name='wq1a'),
                      cst.tile([P, D2], R32, tag='wq1b', name='wq1b')]]
            nc.sync.dma_start(wq_sb[0][0][:], wq[0][0:3, :])
            nc.sync.dma_start(wq_sb[0][1][:], wq[0][3:131, :])
            nc.sync.dma_start(wq_sb[0][2][:], wq[0][131:259, :])
            nc.gpsimd.dma_start(wq_sb[1][0][:], wq[1][0:128, :])
            nc.gpsimd.dma_start(wq_sb[1][1][:], wq[1][128:256, :])

            wk_sb, wv_sb, dm_sb, bi_sb = [], [], [], []
            for lam in range(2):
                wk_sb.append(cst.tile([D2, D2], R32, tag=f'wk{lam}', name=f'wk{lam}'))
                nc.sync.dma_start(wk_sb[lam][:], wk[lam])
                wv_sb.append(cst.tile([D2, D2], R32, tag=f'wv{lam}', name=f'wv{lam}'))
                nc.sync.dma_start(wv_sb[lam][:], wv[lam])
                gt = cst.tile([P, 8], F32, tag=f'gt{lam}', name=f'gt{lam}')
                nc.sync.dma_start(gt[:], gates[lam].rearrange('g d -> d g'))
                dm_sb.append([gt[:, g:g + 1] for g in range(4)])
                bi_sb.append([gt[:, 4 + g:5 + g] for g in range(4)])

            # ---------------- embedding gather + transpose ----------------
            seq_sb = cst.tile([P, 4], I32, tag='seq_sb', name='seq_sb')
            nc.gpsimd.dma_start(seq_sb[:], seq.rearrange('c p -> p c'))
            embT = [cst.tile([P, N], R32, tag=f'embT{t}', name=f'embT{t}')
                    for t in range(2)]
            for c in range(4):
                etok = pre.tile([P, EMB], F32, tag='etok', name='etok')
                nc.gpsimd.indirect_dma_start(
                    out=etok[:], out_offset=None, in_=emb_d,
                    in_offset=bass.IndirectOffsetOnAxis(ap=seq_sb[:, c:c + 1], axis=0))
                for t in range(2):
                    tp = pmm.tile([P, P], F32, tag='mm_ps', name='mm_ps', space='PSUM')
                    nc.tensor.transpose(tp[:], etok[:, t * P:(t + 1) * P], ident[:])
                    if (c + t) % 2 == 0:
                        nc.scalar.copy(embT[t][:, c * P:(c + 1) * P], tp[:])
                    else:
                        nc.vector.tensor_copy(embT[t][:, c * P:(c + 1) * P], tp[:])

            # ---------------- frames: center, covariance, Jacobi eigh ----------------
            x_tm = cst.tile([P, 12], F32, tag='x_tm', name='x_tm')
            nc.sync.dma_start(x_tm[:].rearrange('p (c i) -> p c i', c=4),
                              x_ca.rearrange('(c p) i -> p c i', p=P))
            x_fm = cst.tile([3, N], F32, tag='x_fm', name='x_fm')
            nc.sync.dma_start(x_fm[:], x_ca.rearrange('n i -> i n'))
            mask_col = cst.tile([P, 4], F32, tag='mask_col', name='mask_col')
            nc.gpsimd.dma_start(mask_col[:], maskr[0].rearrange('(c p) -> p c', p=P))
            mask_row = cst.tile([1, N], F32, tag='mask_row', name='mask_row')
            nc.gpsimd.dma_start(mask_row[:], maskr)

            msum_ps = pzt.tile([1, 1], F32, tag='zt_ps', name='zt_ps', space='PSUM')
            mx_ps = pzt.tile([1, 3], F32, tag='zt_ps', name='zt_ps', space='PSUM')
            for c in range(4):
                mm(msum_ps[:], mask_col[:, c:c + 1], ones_col[:], c == 0, c == 3)
                mm(mx_ps[:], mask_col[:, c:c + 1], x_tm[:, 3 * c:3 * c + 3],
                   c == 0, c == 3)
            rmsum = jcb.tile([1, 1], F32, tag='rmsum', name='rmsum')
            nc.vector.reciprocal(rmsum[:], msum_ps[:])
            mx_sb = jcb.tile([1, 3], F32, tag='mx_sb', name='mx_sb')
            nc.scalar.copy(mx_sb[:], mx_ps[:])
            cneg = cst.tile([1, 3], F32, tag='cneg', name='cneg')
            nc.vector.tensor_scalar(cneg[:], mx_sb[:], rmsum[:, 0:1], -1.0,
                                    OP.mult, OP.mult)
            cneg_m = jcb.tile([1, 3], F32, tag='cneg_m', name='cneg_m')
            nc.vector.tensor_scalar(cneg_m[:], cneg[:], msum_ps[:, 0:1], None,
                                    OP.mult)

            c_ps = pmm.tile([3, 3], F32, tag='mm_ps', name='mm_ps', space='PSUM')
            for c in range(4):
                mm(c_ps[:], x_tm[:, 3 * c:3 * c + 3], x_tm[:, 3 * c:3 * c + 3],
                   c == 0, False)
            mm(c_ps[:], cneg[:], mx_sb[:], False, False)
            mm(c_ps[:], mx_sb[:], cneg[:], False, False)
            mm(c_ps[:], cneg[:], cneg_m[:], False, True)

            # Jacobi on flat layouts: a6 = [d0,d1,d2,o01,o02,o12], w9 = V^T rows
            # (compute engines cannot read at a partition offset, so extract the
            # 3x3 entries onto partition 0 via selector-column matmuls)
            c_sb = jcb.tile([3, 3], F32, tag='c_sb', name='c_sb')
            nc.scalar.copy(c_sb[:], c_ps[:])
            crow = []
            for r in range(3):
                row_ps = pmm.tile([1, 3], F32, tag='mm_ps', name='mm_ps',
                                  space='PSUM')
                mm(row_ps[:], ident[0:3, r:r + 1], c_sb[:], True, True)
                cr = jcb.tile([1, 3], F32, tag=f'crow{r}', name=f'crow{r}')
                nc.scalar.copy(cr[:], row_ps[:])
                crow.append(cr)
            a6 = jcb.tile([1, 6], F32, tag='a6', name='a6')
            for (k, (r_, c_)) in enumerate([(0, 0), (1, 1), (2, 2), (0, 1), (0, 2),
                                            (1, 2)]):
                nc.scalar.copy(a6[:, k:k + 1], crow[r_][:, c_:c_ + 1])
            w9 = jcb.tile([1, 9], F32, tag='w9', name='w9')
            nc.vector.memset(w9[:], 0.0)
            for i in range(3):
                nc.vector.memset(w9[:, 4 * i:4 * i + 1], 1.0)

            OIDX = {(0, 1): 3, (0, 2): 4, (1, 2): 5}
            V = nc.vector

            def j1(name):
                return jcb.tile([1, 1], F32, tag=f'j1_{name}', name=f'j1_{name}')

            def j3(name):
                return jcb.tile([1, 3], F32, tag=f'j3_{name}', name=f'j3_{name}')

            for _s in range(SWEEPS):
                for (p_, q_) in [(0, 1), (0, 2), (1, 2)]:
                    apq = a6[:, OIDX[(p_, q_)]:OIDX[(p_, q_)] + 1]
                    dp = a6[:, p_:p_ + 1]
                    dq = a6[:, q_:q_ + 1]
                    half = j1('half')
                    V.tensor_tensor(out=half[:], in0=dq, in1=dp, op=OP.subtract)
                    hsc = j1('hsc')
                    V.tensor_scalar(hsc[:], half[:], 0.5, None, OP.mult)
                    rapq0 = j1('rapq0')
                    V.reciprocal(rapq0[:], apq)
                    rapq = j1('rapq')
                    V.tensor_scalar(rapq[:], rapq0[:], -1e30, 1e30, OP.max, OP.min)
                    th0 = j1('th0')
                    V.tensor_tensor(out=th0[:], in0=hsc[:], in1=rapq[:], op=OP.mult)
                    th = j1('th')
                    V.tensor_scalar(th[:], th0[:], -1e17, 1e17, OP.max, OP.min)
                    th2 = j1('th2')
                    V.tensor_tensor(out=th2[:], in0=th[:], in1=th[:], op=OP.mult)
                    rt = j1('rt')
                    nc.scalar.activation(rt[:], th2[:], AF.Sqrt, bias=1.0)
                    thneg = j1('thneg')
                    V.tensor_scalar(thneg[:], th[:], -1.0, None, OP.mult)
                    absth = j1('absth')
                    V.tensor_tensor(out=absth[:], in0=th[:], in1=thneg[:], op=OP.max)
                    den = j1('den')
                    V.tensor_tensor(out=den[:], in0=absth[:], in1=rt[:], op=OP.add)
                    ge = j1('ge')
                    V.tensor_scalar(ge[:], th[:], 0.0, None, OP.is_ge)
                    sgn = j1('sgn')
                    V.tensor_scalar(sgn[:], ge[:], 2.0, -1.0, OP.mult, OP.add)
                    rden = j1('rden')
                    V.reciprocal(rden[:], den[:])
                    t_ = j1('t_')
                    V.tensor_tensor(out=t_[:], in0=sgn[:], in1=rden[:], op=OP.mult)
                    t2 = j1('t2')
                    V.tensor_tensor(out=t2[:], in0=t_[:], in1=t_[:], op=OP.mult)
                    rt2 = j1('rt2')
                    nc.scalar.activation(rt2[:], t2[:], AF.Sqrt, bias=1.0)
                    cc = j1('cc')
                    V.reciprocal(cc[:], rt2[:])
                    ss = j1('ss')
                    V.tensor_tensor(out=ss[:], in0=t_[:], in1=cc[:], op=OP.mult)
                    tneg = j1('tneg')
                    V.tensor_scalar(tneg[:], t_[:], -1.0, None, OP.mult)
                    ssneg = j1('ssneg')
                    V.tensor_scalar(ssneg[:], ss[:], -1.0, None, OP.mult)
                    # diagonal updates (in place; reads precede the writes)
                    V.scalar_tensor_tensor(out=dp, in0=apq, scalar=tneg[:, 0:1],
                                           in1=dp, op0=OP.mult, op1=OP.add)
                    V.scalar_tensor_tensor(out=dq, in0=apq, scalar=t_[:, 0:1],
                                           in1=dq, op0=OP.mult, op1=OP.add)
                    V.memset(apq, 0.0)
                    # off-diagonal pair (p,r), (q,r)
                    r_ = 3 - p_ - q_
                    kp = OIDX[(min(p_, r_), max(p_, r_))]
                    kq = OIDX[(min(q_, r_), max(q_, r_))]
                    apr = a6[:, kp:kp + 1]
                    aqr = a6[:, kq:kq + 1]
                    x1 = j1('x1')
                    V.tensor_scalar(x1[:], apr, cc[:, 0:1], None, OP.mult)
                    x2 = j1('x2')
                    V.tensor_scalar(x2[:], apr, ss[:, 0:1], None, OP.mult)
                    V.scalar_tensor_tensor(out=apr, in0=aqr, scalar=ssneg[:, 0:1],
                                           in1=x1[:], op0=OP.mult, op1=OP.add)
                    V.scalar_tensor_tensor(out=aqr, in0=aqr, scalar=cc[:, 0:1],
                                           in1=x2[:], op0=OP.mult, op1=OP.add)
                    # eigenvector rows of V^T
                    wp = w9[:, 3 * p_:3 * p_ + 3]
                    wqr = w9[:, 3 * q_:3 * q_ + 3]
                    y1 = j3('y1')
                    V.tensor_scalar(y1[:], wp, cc[:, 0:1], None, OP.mult)
                    y2 = j3('y2')
                    V.tensor_scalar(y2[:], wp, ss[:, 0:1], None, OP.mult)
                    V.scalar_tensor_tensor(out=wp, in0=wqr, scalar=ssneg[:, 0:1],
                                           in1=y1[:], op0=OP.mult, op1=OP.add)
                    V.scalar_tensor_tensor(out=wqr, in0=wqr, scalar=cc[:, 0:1],
                                           in1=y2[:], op0=OP.mult, op1=OP.add)

            # ascending eigenvalue sort (3-element network)
            for (ai, bi_) in [(0, 1), (0, 2), (1, 2)]:
                da = a6[:, ai:ai + 1]
                db = a6[:, bi_:bi_ + 1]
                cmp = jcb.tile([1, 1], I32, tag='j1_cmp', name='j1_cmp')
                V.tensor_tensor(out=cmp[:], in0=da, in1=db, op=OP.is_le)
                dlo = j1('dlo')
                V.tensor_tensor(out=dlo[:], in0=da, in1=db, op=OP.min)
                dhi = j1('dhi')
                V.tensor_tensor(out=dhi[:], in0=da, in1=db, op=OP.max)
                V.tensor_copy(da, dlo[:])
                V.tensor_copy(db, dhi[:])
                wa = w9[:, 3 * ai:3 * ai + 3]
                wb = w9[:, 3 * bi_:3 * bi_ + 3]
                wlo = j3('wlo')
                V.select(out=wlo[:], mask=cmp[:].to_broadcast([1, 3]),
                         on_true=wa, on_false=wb)
                whi = j3('whi')
                V.select(out=whi[:], mask=cmp[:].to_broadcast([1, 3]),
                         on_true=wb, on_false=wa)
                V.tensor_copy(wa, wlo[:])
                V.tensor_copy(wb, whi[:])

            # spread w9 (1,9) across partitions -> (3,3) via DRAM roundtrip
            nc.sync.dma_start(jac_scr[0:9].rearrange('(a n) -> a n', a=1), w9[:])
            vt_f = jcb.tile([3, 3], F32, tag='vt_f', name='vt_f')
            nc.sync.dma_start(vt_f[:], jac_scr[0:9].rearrange('(r c) -> r c', r=3))
            vt_sb = cst.tile([3, 3], F32, tag='vt_sb', name='vt_sb')
            nc.scalar.copy(vt_sb[:], vt_f[:])

            # F_all (3, 24), Xc_fm (3, N), hX per frame -> DRAM scratch
            f_ps = pmm.tile([3, 24], F32, tag='mm_ps', name='mm_ps', space='PSUM')
            mm(f_ps[:], vt_sb[:], s_sb[:], True, True)
            fa_sb = cst.tile([3, 24], F32, tag='fa_sb', name='fa_sb')
            nc.scalar.copy(fa_sb[:], f_ps[:])

            xc_ps = pau.tile([3, N], F32, tag='au_ps', name='au_ps', space='PSUM')
            mm(xc_ps[:], ident[0:3, 0:3], x_fm[:], True, False)
            mm(xc_ps[:], cneg[:], mask_row[:], False, True)
            xc_fm = cst.tile([3, N], F32, tag='xc_fm', name='xc_fm')
            nc.scalar.copy(xc_fm[:], xc_ps[:])

            for o in range(NF):
                hx_ps = pmm.tile([3, N], F32, tag='mm_ps', name='mm_ps', space='PSUM')
                mm(hx_ps[:], fa_sb[:, 3 * o:3 * o + 3], xc_fm[:], True, True)
                hx_st = pre.tile([3, N], R32, tag='hx_st', name='hx_st')
                if o % 2 == 0:
                    nc.scalar.copy(hx_st[:], hx_ps[:])
                else:
                    nc.vector.tensor_copy(hx_st[:], hx_ps[:])
                nc.gpsimd.dma_start(hx_scr[o], hx_st[:])

            # ---------------- layers ----------------
            h2 = {}
            for lam in range(2):
                for d in range(2):
                    h2[(lam, d)] = h2p.tile([P, NF * N], R32, tag=f'h2_{lam}_{d}',
                                            name=f'h2_{lam}_{d}')

            wu_lam = [None, None]

            for lam in range(2):
                wu_lam[lam] = cst.tile([D2, 8 * D2], R32, tag='wu_sb', name='wu_sb')
                nc.sync.dma_start(wu_lam[lam][:], wu[lam])

                def attention(f):
                    # ---- attention + U for frame f ----
                    if lam == 0:
                        hx_f = att.tile([3, N], R32, tag='hx_f', name='hx_f')
                        nc.gpsimd.dma_start(hx_f[:], hx_scr[f])
                        parts = [(wq_sb[0][0], hx_f[:]),
                                 (wq_sb[0][1], embT[0][:]),
                                 (wq_sb[0][2], embT[1][:])]
                    else:
                        parts = [(wq_sb[1][0], h2[(0, 0)][:, f * N:(f + 1) * N]),
                                 (wq_sb[1][1], h2[(0, 1)][:, f * N:(f + 1) * N])]

                    q_ps = pmm.tile([P, N], F32, tag='mm_ps', name='mm_ps',
                                    space='PSUM')
                    for i, (w_t, x_ap) in enumerate(parts):
                        mm(q_ps[:], w_t[:], x_ap, i == 0, i == len(parts) - 1)
                    q_sb = att.tile([P, N], R32, tag='q_sb', name='q_sb')
                    nc.scalar.copy(q_sb[:], q_ps[:])

                    k_ps = pmm.tile([P, N], F32, tag='mm_ps', name='mm_ps',
                                    space='PSUM')
                    mm(k_ps[:], wk_sb[lam][:], q_sb[:], True, True)
                    k_sb = att.tile([P, N], R32, tag='k_sb', name='k_sb')
                    nc.vector.tensor_copy(k_sb[:], k_ps[:])

                    q_tm = att.tile([P, N], R32, tag='q_tm', name='q_tm')
                    for m in range(4):
                        tpr = pmm.tile([P, P], R32, tag='mm_ps', name='mm_ps',
                                       space='PSUM')
                        nc.tensor.transpose(tpr[:], q_sb[:, m * P:(m + 1) * P],
                                            ident_r[:])
                        if m % 2 == 0:
                            nc.scalar.copy(q_tm[:, m * P:(m + 1) * P], f32(tpr[:]))
                        else:
                            nc.vector.tensor_copy(q_tm[:, m * P:(m + 1) * P],
                                                  f32(tpr[:]))

                    z_ps = pzt.tile([1, N], F32, tag='zt_ps', name='zt_ps',
                                    space='PSUM')
                    t_ps = pzt.tile([P, N], F32, tag='zt_ps', name='zt_ps',
                                    space='PSUM')
                    for m in range(4):
                        s_ps = psc.tile([P, N], F32, tag='s_ps', name='s_ps',
                                        space='PSUM')
                        mm(s_ps[:], k_sb[:, m * P:(m + 1) * P], q_sb[:], True, True)
                        e_sb = att.tile([P, N], R32, tag='e_sb', name='e_sb')
                        nc.scalar.activation(e_sb[:], s_ps[:], AF.Exp, scale=SCALE)
                        mm(z_ps[:], ones_col_r[:], e_sb[:], m == 0, m == 3)
                        mm(t_ps[:], q_tm[:, m * P:(m + 1) * P], e_sb[:], m == 0,
                           m == 3)
                    recip = att.tile([1, N], F32, tag='recip', name='recip')
                    nc.vector.reciprocal(recip[:], z_ps[:])
                    recip_r = att.tile([1, N], R32, tag='recip_r', name='recip_r')
                    nc.vector.tensor_copy(recip_r[:], recip[:])
                    rb_ps = pmm.tile([P, N], F32, tag='mm_ps', name='mm_ps',
                                     space='PSUM')
                    mm(rb_ps[:], ones_row_r[:], recip_r[:], True, True)
                    rb_sb = att.tile([P, N], F32, tag='rb_sb', name='rb_sb')
                    nc.scalar.copy(rb_sb[:], rb_ps[:])
                    tn_sb = att.tile([P, N], R32, tag='tn_sb', name='tn_sb')
                    nc.vector.tensor_tensor(out=tn_sb[:], in0=t_ps[:], in1=rb_sb[:],
                                            op=OP.mult)

                    a_ps = pau.tile([P, N], F32, tag='au_ps', name='au_ps',
                                    space='PSUM')
                    mm(a_ps[:], wv_sb[lam][:], tn_sb[:], True, False)
                    mm(a_ps[:], ident_r[:], q_sb[:], False, True)
                    a_sb = att.tile([P, N], R32, tag='a_sb', name='a_sb')
                    nc.scalar.copy(a_sb[:], a_ps[:])

                    u_fr = ust.tile([P, 8 * N], R32, tag='u_fr', name='u_fr')
                    for j in range(8):
                        u_ps = pau.tile([P, N], F32, tag='au_ps', name='au_ps',
                                        space='PSUM')
                        mm(u_ps[:], wu_lam[lam][:, j * P:(j + 1) * P], a_sb[:],
                           True, True)
                        u_sl = u_fr[:, j * N:(j + 1) * N]
                        dd = j // 4
                        mt = j % 4
                        if mt in (1, 2):
                            nc.scalar.activation(
                                u_sl, u_ps[:], AF.Identity,
                                bias=bi_sb[lam][dd * 2 + (mt - 1)])
                        elif j % 3 == 0:
                            nc.scalar.copy(u_sl, u_ps[:])
                        else:
                            nc.vector.tensor_copy(u_sl, u_ps[:])

                    return u_fr

                def scan_lanes(f, u_fr):
                    # ---- SRU scan lanes for frame f ----
                    for d in range(2):
                        rev = (d == 1)

                        def rv(ap):
                            return ap[:, ::-1] if rev else ap

                        u0 = u_fr[:, (d * 4 + 0) * N:(d * 4 + 1) * N]
                        u1 = u_fr[:, (d * 4 + 1) * N:(d * 4 + 2) * N]

                        c_buf = sc2.tile([P, N + 1], R32, tag='c_buf', name='c_buf')
                        nc.scalar.copy(c_buf[:, 0:1], zcol[:])

                        # pass 0: gates from c=0
                        f_t = sc2.tile([P, N], F32, tag='f_t', name='f_t')
                        nc.scalar.activation(f_t[:], rv(f32(u1)), AF.Sigmoid)
                        x_t = sc2.tile([P, N], F32, tag='x_t', name='x_t')
                        nc.vector.scalar_tensor_tensor(
                            out=x_t[:], in0=f_t[:], scalar=1.0, in1=rv(f32(u0)),
                            op0=OP.subtract, op1=OP.mult)
                        nc.vector.tensor_tensor_scan(
                            out=c_buf[:, 1:N + 1], data0=f_t[:], data1=x_t[:],
                            initial=0.0, op0=OP.mult, op1=OP.subtract)

                        for _it in range(NMID + 1):
                            ag_t = sc2.tile([P, N], F32, tag='ag_t', name='ag_t')
                            nc.vector.scalar_tensor_tensor(
                                out=ag_t[:], in0=f32(c_buf[:, 0:N]),
                                scalar=dm_sb[lam][0 * 2 + d], in1=rv(f32(u1)),
                                op0=OP.mult, op1=OP.add)
                            f_t = sc2.tile([P, N], F32, tag='f_t', name='f_t')
                            nc.scalar.activation(f_t[:], ag_t[:], AF.Sigmoid)
                            x_t = sc2.tile([P, N], F32, tag='x_t', name='x_t')
                            nc.vector.scalar_tensor_tensor(
                                out=x_t[:], in0=f_t[:], scalar=1.0, in1=rv(f32(u0)),
                                op0=OP.subtract, op1=OP.mult)
                            nc.vector.tensor_tensor_scan(
                                out=c_buf[:, 1:N + 1], data0=f_t[:], data1=x_t[:],
                                initial=0.0, op0=OP.mult, op1=OP.subtract)

                        u2 = u_fr[:, (d * 4 + 2) * N:(d * 4 + 3) * N]
                        u3 = u_fr[:, (d * 4 + 3) * N:(d * 4 + 4) * N]

                        ar_t = sc2.tile([P, N], F32, tag='ar_t', name='ar_t')
                        nc.vector.scalar_tensor_tensor(
                            out=ar_t[:], in0=f32(c_buf[:, 0:N]),
                            scalar=dm_sb[lam][1 * 2 + d], in1=rv(f32(u2)),
                            op0=OP.mult, op1=OP.add)
                        r_t = sc2.tile([P, N], F32, tag='r_t', name='r_t')
                        nc.scalar.activation(r_t[:], ar_t[:], AF.Sigmoid)

                        d_t = sc2.tile([P, N], F32, tag='d_t', name='d_t')
                        nc.vector.tensor_tensor(out=d_t[:], in0=f32(c_buf[:, 1:N + 1]),
                                                in1=rv(f32(u3)), op=OP.subtract)
                        m_t = sc2.tile([P, N], F32, tag='m_t', name='m_t')
                        nc.vector.tensor_tensor(out=m_t[:], in0=r_t[:], in1=d_t[:],
                                                op=OP.mult)
                        dst = h2[(lam, d)][:, f * N:(f + 1) * N]
                        nc.vector.tensor_tensor(out=rv(dst), in0=m_t[:],
                                                in1=rv(f32(u3)), op=OP.add)



                for f in range(NF):
                    scan_lanes(f, attention(f))
            # ---------------- output: mean over frames + transpose ----------------
            m_fm = [pre.tile([P, N], F32, tag=f'm_fm{t}', name=f'm_fm{t}')
                    for t in range(2)]
            for t in range(2):
                mview = f32(h2[(1, t)][:]).rearrange('p (f l) -> p l f', f=NF)
                nc.vector.tensor_reduce(out=m_fm[t][:], in_=mview,
                                        axis=mybir.AxisListType.X, op=OP.add)
            for c in range(4):
                o_st = pre.tile([P, H], F32, tag='o_st', name='o_st')
                for t in range(2):
                    tp = pmm.tile([P, P], F32, tag='mm_ps', name='mm_ps', space='PSUM')
                    nc.tensor.transpose(tp[:], m_fm[t][:, c * P:(c + 1) * P], ident[:])
                    nc.scalar.activation(o_st[:, t * P:(t + 1) * P], tp[:], AF.Copy,
                                         scale=1.0 / NF)
                nc.sync.dma_start(out_d[c * P:(c + 1) * P, :], o_st[:])

        if n_iter == 1:
            body()
        else:
            with tc.For_i(0, n_iter, 1):
                body()

    nc.compile()
    return nc


_CACHE = {}


def _get_nc(n_iter=1):
    if n_iter not in _CACHE:
        _CACHE[n_iter] = _build(n_iter)
    return _CACHE[n_iter]


def host_inputs(inputs):
    """Build the 8 per-core input maps (pure slicing/packing, no math)."""
    ops = np.array([[i, j, k] for i in (-1, 1) for j in (-1, 1) for k in (-1, 1)],
                   np.float32)
    S = np.zeros((3, 24), np.float32)
    for o in range(8):
        S[:, 3 * o:3 * o + 3] = np.diag(ops[o])

    shared = {'emb': np.ascontiguousarray(inputs['emb'], np.float32), 'sops': S}
    for lam in range(2):
        shared[f'wq{lam}'] = np.ascontiguousarray(inputs[f'Wq{lam}'], np.float32)
        shared[f'wk{lam}'] = np.ascontiguousarray(inputs[f'Wk{lam}'], np.float32)
        shared[f'wv{lam}'] = np.ascontiguousarray(inputs[f'Wv{lam}'], np.float32)
        shared[f'wu{lam}'] = np.ascontiguousarray(inputs[f'Wu{lam}'], np.float32)
        shared[f'gates{lam}'] = np.ascontiguousarray(np.concatenate(
            [inputs[f'vf{lam}'], inputs[f'vr{lam}'],
             inputs[f'bf{lam}'], inputs[f'br{lam}']]), np.float32)


    in_maps = []
    for b in range(B):
        m = dict(shared)
        m['x_ca'] = np.ascontiguousarray(inputs['noisy_cords'][b, :, 1, :], np.float32)
        m['maskr'] = np.ascontiguousarray(
            np.asarray(inputs['mask'][b], np.float32).reshape(1, N))
        m['seq'] = np.ascontiguousarray(
            np.asarray(inputs['noisy_seqs'][b], np.int32).reshape(4, P))
        in_maps.append(m)
    return in_maps


def kernel(**inputs):
    nc = _get_nc(1)
    in_maps = host_inputs(inputs)
    res = run_bass_kernel_spmd(nc, in_maps, core_ids=list(range(B)))
    return np.stack([res.results[b]['out'] for b in range(B)], axis=0)



# revision 34
# speedup vs baseline: 1.2353x; 1.2353x over previous
"""Trainium2 Bass kernel for nn_FAEncoder: frame-averaged SRU++ encoder.

Sharding: data-parallel over B (8 samples -> 8 cores). Each core processes its
sample's 8 sign-flip frames end to end; weights are replicated. No collectives.

v2 design notes (vs the fp32r baseline):
- fp16 on the whole matmul path (PSUM accumulation stays fp32). This unlocks
  the DVE 2x/4x perf modes for tensor_scalar / tensor_tensor / copies.
- Gates use tanh instead of sigmoid: sigmoid(x) = 0.5*tanh(x/2)+0.5. exp and
  tanh live in the same activation-function table, so the ACT engine stops
  thrashing table loads between softmax and SRU gates (~1.3us per reload).
- Fixed point runs 2 passes total (pass0 + 1 refinement); method error vs the
  exact recurrence is ~1e-4 on this data distribution.
- SRU scan sections operate on frame-batched tiles [P, FBLK*N]: per-frame
  carry reset is done by zeroing the scan multiplier f at frame-start columns
  (strided memset), and zeroing the c_prev contribution in gate args.
- Embedding rows are gathered host-side (pure indexing) and shipped
  pre-transposed; the frame-invariant part of layer-0 q is computed once.
- hX for all frames lives in SBUF ([3, NF*N]); no DRAM scratch roundtrip.
- Engine assignment of copies/elementwise ops is table-driven (ASSIGN) and was
  tuned against the instruction-cost timeline simulator.
"""
import os
import sys
from contextlib import ExitStack

import numpy as np

for _p in ('/opt/trn_rl_repo', '/root/.axon_site/_ro/trn_rl_repo'):
    if os.path.isdir(_p) and _p not in sys.path:
        sys.path.append(_p)

import concourse.bacc as bacc
import concourse.bass as bass
import concourse.mybir as mybir
from concourse.bass_utils import run_bass_kernel_spmd
from concourse.masks import make_identity
from concourse.tile import TileContext

F32 = mybir.dt.float32
F16 = mybir.dt.float16
I32 = mybir.dt.int32
AF = mybir.ActivationFunctionType
OP = mybir.AluOpType

B, N, H, D2, EMB, VOCAB = 8, 512, 256, 128, 256, 100
NF, ND = 8, 2
P = 128
SCALE = float(1.0 / np.sqrt(np.float32(D2)))
SWEEPS = 3    # cyclic Jacobi sweeps for the 3x3 eigh
FBLK = 2      # frames per batched scan op
NBLK = NF // FBLK
CW = FBLK * N  # batched op width
SCN_BUFS = 1  # buffers for scan temp tiles
UNROLL = False  # python-unroll the timing loop (for TimelineSim)

# Engine assignment per op-site: 'a' = scalar/ACT, 'v' = vector/DVE,
# 'p' = gpsimd/Pool. Tuned against TimelineSim.
# NOTE: Pool/GPSIMD cannot access PSUM -- any op reading PSUM must be 'a'/'v'.
ASSIGN = {
    'hx': 'av',       # rotation for the 8 hX copies (PSUM)
    'embq': 'a',      # q_emb copy (PSUM)
    'q': 'a',         # PSUM
    'k': 'v',         # PSUM
    'qtm': 'v',       # merged q-transpose copy (fp16 PSUM -> 2x mode)
    'rb': 'a',        # PSUM
    'tn': 'v',        # PSUM operand
    'acp': 'a',       # PSUM
    'u0': 'v',        # PSUM (ts scale 0.5 -> u0h)
    'u1': 'a',        # PSUM
    'u2': 'a',        # PSUM
    'u3': 'v',        # PSUM
    # scan section (per chunk, all SBUF-only)
    'f': 'v',
    'x': 'v',
    'cv': 'v',
    'ag': 'p',
    'ar': 'p',
    'D': 'v',
    'r': 'v',
    'm': 'v',
    'hf': 'v',
    'mean': 'v',
}


def _build(n_iter=1):
    nc = bacc.Bacc('TRN2', num_devices=B, debug=False)

    def eng(key, idx=0):
        c = ASSIGN[key]
        c = c[idx % len(c)]
        return {'a': nc.scalar, 'v': nc.vector, 'p': nc.gpsimd}[c]

    def copy_on(e, out, in_):
        if e is nc.scalar:
            e.copy(out, in_)
        else:
            e.tensor_copy(out, in_)

    def mm(out, lhsT, rhs, start, stop):
        nc.tensor.matmul(out, lhsT, rhs, start=start, stop=stop)

    dt = nc.dram_tensor
    x_ca = dt('x_ca', [N, 3], F32, kind='ExternalInput').ap()
    maskr = dt('maskr', [1, N], F32, kind='ExternalInput').ap()
    embt = dt('embt', [EMB, N], F16, kind='ExternalInput').ap()
    sops = dt('sops', [3, 24], F32, kind='ExternalInput').ap()
    wq = [dt('wq0', [3 + EMB, D2], F16, kind='ExternalInput').ap(),
          dt('wq1', [H, D2], F16, kind='ExternalInput').ap()]
    wk = [dt(f'wk{i}', [D2, D2], F16, kind='ExternalInput').ap() for i in range(2)]
    wv = [dt(f'wv{i}', [D2, D2], F16, kind='ExternalInput').ap() for i in range(2)]
    wu = [dt(f'wu{i}', [D2, 8 * D2], F16, kind='ExternalInput').ap() for i in range(2)]
    gates = [dt(f'gates{i}', [8, P], F32, kind='ExternalInput').ap() for i in range(2)]
    out_d = dt('out', [N, H], F32, kind='ExternalOutput').ap()

    jac_scr = dt('jac_scr', [16], F32, kind='Internal').ap()

    with TileContext(nc) as tc, ExitStack() as es:
        cst = es.enter_context(tc.tile_pool(name='cst', bufs=1))
        geo = es.enter_context(tc.tile_pool(name='geo', bufs=2))
        pre = es.enter_context(tc.tile_pool(name='pre', bufs=2))
        mnp = es.enter_context(tc.tile_pool(name='mnp', bufs=1))
        att = es.enter_context(tc.tile_pool(name='att', bufs=2))
        ust = es.enter_context(tc.tile_pool(name='ust', bufs=1))
        scn = es.enter_context(tc.tile_pool(name='scn', bufs=SCN_BUFS))
        sc2 = es.enter_context(tc.tile_pool(name='sc2', bufs=1))
        h2p = es.enter_context(tc.tile_pool(name='h2p', bufs=1))
        h2q = es.enter_context(tc.tile_pool(name='h2q', bufs=1))
        jcb = es.enter_context(tc.tile_pool(name='jcb', bufs=2))
        pmm = es.enter_context(tc.tile_pool(name='pmm', bufs=2, space='PSUM'))
        psc = es.enter_context(tc.tile_pool(name='psc', bufs=2, space='PSUM'))
        pzt = es.enter_context(tc.tile_pool(name='pzt', bufs=2, space='PSUM'))
        pau = es.enter_context(tc.tile_pool(name='pau', bufs=2, space='PSUM'))

        def init_static():
            # one-time constants + persistent tiles (emitted once, pre-loop)
            st = {}
            ident = cst.tile([P, P], F32, tag='ident', name='ident')
            make_identity(nc, ident)
            ident_h = cst.tile([P, P], F16, tag='ident_h', name='ident_h')
            nc.scalar.copy(ident_h[:], ident[:])
            ones_col = cst.tile([P, 1], F16, tag='ones_col', name='ones_col')
            nc.vector.memset(ones_col[:], 1.0)
            ones_row = cst.tile([1, P], F16, tag='ones_row', name='ones_row')
            nc.vector.memset(ones_row[:], 1.0)
            ones_col_f = cst.tile([P, 1], F32, tag='ones_col_f',
                                  name='ones_col_f')
            nc.vector.memset(ones_col_f[:], 1.0)
            st['ident'], st['ident_h'] = ident, ident_h
            st['ones_col'], st['ones_row'] = ones_col, ones_row
            st['ones_col_f'] = ones_col_f
            h2 = {}
            for lam in range(2):
                for d in range(2):
                    pool = h2q if lam == 1 else h2p
                    h2[(lam, d)] = pool.tile([P, NF * N], F16,
                                             tag=f'h2_{lam}_{d}',
                                             name=f'h2_{lam}_{d}')
            st['h2'] = h2
            return st

        def body():
            ident = ST['ident']
            ident_h = ST['ident_h']
            ones_col = ST['ones_col']
            ones_row = ST['ones_row']
            ones_col_f = ST['ones_col_f']
            h2 = ST['h2']
            # ---------------- geometry inputs first (gate the serial eigh) ---
            x_tm = geo.tile([P, 12], F32, tag='x_tm', name='x_tm')
            nc.sync.dma_start(x_tm[:].rearrange('p (c i) -> p c i', c=4),
                              x_ca.rearrange('(c p) i -> p c i', p=P))
            x_fm = geo.tile([3, N], F32, tag='x_fm', name='x_fm')
            nc.sync.dma_start(x_fm[:], x_ca.rearrange('n i -> i n'))
            mask_col = geo.tile([P, 4], F32, tag='mask_col', name='mask_col')
            nc.sync.dma_start(mask_col[:], maskr[0].rearrange('(c p) -> p c', p=P))
            mask_row = cst.tile([1, N], F32, tag='mask_row', name='mask_row')
            nc.sync.dma_start(mask_row[:], maskr)
            s_sb = cst.tile([3, 24], F32, tag='s_sb', name='s_sb')
            nc.sync.dma_start(s_sb[:], sops)

            # ---------------- weight staging ----------------
            embT = [cst.tile([P, N], F16, tag=f'embT{t}', name=f'embT{t}')
                    for t in range(2)]
            for t in range(2):
                nc.sync.dma_start(embT[t][:], embt[t * P:(t + 1) * P, :])

            wq_sb = [[cst.tile([3, D2], F16, tag='wq0a', name='wq0a'),
                      cst.tile([P, D2], F16, tag='wq0b', name='wq0b'),
                      cst.tile([P, D2], F16, tag='wq0c', name='wq0c')],
                     [cst.tile([P, D2], F16, tag='wq1a', name='wq1a'),
                      cst.tile([P, D2], F16, tag='wq1b', name='wq1b')]]
            nc.sync.dma_start(wq_sb[0][0][:], wq[0][0:3, :])
            nc.sync.dma_start(wq_sb[0][1][:], wq[0][3:131, :])
            nc.sync.dma_start(wq_sb[0][2][:], wq[0][131:259, :])
            nc.sync.dma_start(wq_sb[1][0][:], wq[1][0:128, :])
            nc.sync.dma_start(wq_sb[1][1][:], wq[1][128:256, :])

            wk_sb, wv_sb, wu_sb, gt_sb, gth_sb = [], [], [], [], []
            for lam in range(2):
                wk_sb.append(cst.tile([D2, D2], F16, tag=f'wk{lam}', name=f'wk{lam}'))
                nc.sync.dma_start(wk_sb[lam][:], wk[lam])
                wv_sb.append(cst.tile([D2, D2], F16, tag=f'wv{lam}', name=f'wv{lam}'))
                nc.sync.dma_start(wv_sb[lam][:], wv[lam])
                wu_sb.append(cst.tile([D2, 8 * D2], F16, tag=f'wu{lam}',
                                      name=f'wu{lam}'))
                nc.sync.dma_start(wu_sb[lam][:], wu[lam])
                gt = cst.tile([P, 8], F32, tag=f'gt{lam}', name=f'gt{lam}')
                nc.sync.dma_start(gt[:], gates[lam].rearrange('g d -> d g'))
                gth = cst.tile([P, 8], F32, tag=f'gth{lam}', name=f'gth{lam}')
                nc.vector.tensor_scalar(gth[:], gt[:], 0.5, None, OP.mult)
                gt_sb.append(gt)
                gth_sb.append(gth)

            def vfh(lam, d):
                return gth_sb[lam][:, d:d + 1]

            def vrh(lam, d):
                return gth_sb[lam][:, 2 + d:3 + d]

            def bfh(lam, d):
                return gth_sb[lam][:, 4 + d:5 + d]

            def brh(lam, d):
                return gth_sb[lam][:, 6 + d:7 + d]

            # ---------------- frames: center, covariance, Jacobi eigh ----------------

            msum_ps = pzt.tile([1, 1], F32, tag='zt_ps', name='zt_ps', space='PSUM')
            mx_ps = pzt.tile([1, 3], F32, tag='zt_ps', name='zt_ps', space='PSUM')
            for c in range(4):
                mm(msum_ps[:], mask_col[:, c:c + 1], ones_col_f[:], c == 0, c == 3)
                mm(mx_ps[:], mask_col[:, c:c + 1], x_tm[:, 3 * c:3 * c + 3],
                   c == 0, c == 3)
            rmsum = jcb.tile([1, 1], F32, tag='rmsum', name='rmsum')
            nc.vector.reciprocal(rmsum[:], msum_ps[:])
            mx_sb = jcb.tile([1, 3], F32, tag='mx_sb', name='mx_sb')
            nc.scalar.copy(mx_sb[:], mx_ps[:])
            cneg = geo.tile([1, 3], F32, tag='cneg', name='cneg')
            nc.vector.tensor_scalar(cneg[:], mx_sb[:], rmsum[:, 0:1], -1.0,
                                    OP.mult, OP.mult)
            cneg_m = jcb.tile([1, 3], F32, tag='cneg_m', name='cneg_m')
            nc.vector.tensor_scalar(cneg_m[:], cneg[:], msum_ps[:, 0:1], None,
                                    OP.mult)

            c_ps = pmm.tile([3, 3], F32, tag='mm_ps', name='mm_ps', space='PSUM')
            for c in range(4):
                mm(c_ps[:], x_tm[:, 3 * c:3 * c + 3], x_tm[:, 3 * c:3 * c + 3],
                   c == 0, False)
            mm(c_ps[:], cneg[:], mx_sb[:], False, False)
            mm(c_ps[:], mx_sb[:], cneg[:], False, False)
            mm(c_ps[:], cneg[:], cneg_m[:], False, True)

            # Jacobi on flat layouts: a6 = [d0,d1,d2,o01,o02,o12], w9 = V^T rows
            c_sb = jcb.tile([3, 3], F32, tag='c_sb', name='c_sb')
            nc.scalar.copy(c_sb[:], c_ps[:])
            crow = []
            for r in range(3):
                row_ps = pmm.tile([1, 3], F32, tag='mm_ps', name='mm_ps',
                                  space='PSUM')
                mm(row_ps[:], ident[0:3, r:r + 1], c_sb[:], True, True)
                cr = jcb.tile([1, 3], F32, tag=f'crow{r}', name=f'crow{r}')
                nc.scalar.copy(cr[:], row_ps[:])
                crow.append(cr)
            a6 = jcb.tile([1, 6], F32, tag='a6', name='a6')
            for (k, (r_, c_)) in enumerate([(0, 0), (1, 1), (2, 2), (0, 1), (0, 2),
                                            (1, 2)]):
                nc.scalar.copy(a6[:, k:k + 1], crow[r_][:, c_:c_ + 1])
            w9 = jcb.tile([1, 9], F32, tag='w9', name='w9')
            nc.vector.memset(w9[:], 0.0)
            for i in range(3):
                nc.vector.memset(w9[:, 4 * i:4 * i + 1], 1.0)

            OIDX = {(0, 1): 3, (0, 2): 4, (1, 2): 5}
            V = nc.vector

            def j1(name):
                return jcb.tile([1, 1], F32, tag=f'j1_{name}', name=f'j1_{name}')

            def j3(name):
                return jcb.tile([1, 3], F32, tag=f'j3_{name}', name=f'j3_{name}')

            for _s in range(SWEEPS):
                for (p_, q_) in [(0, 1), (0, 2), (1, 2)]:
                    apq = a6[:, OIDX[(p_, q_)]:OIDX[(p_, q_)] + 1]
                    dp = a6[:, p_:p_ + 1]
                    dq = a6[:, q_:q_ + 1]
                    half = j1('half')
                    V.tensor_tensor(out=half[:], in0=dq, in1=dp, op=OP.subtract)
                    hsc = j1('hsc')
                    V.tensor_scalar(hsc[:], half[:], 0.5, None, OP.mult)
                    rapq0 = j1('rapq0')
                    V.reciprocal(rapq0[:], apq)
                    rapq = j1('rapq')
                    V.tensor_scalar(rapq[:], rapq0[:], -1e30, 1e30, OP.max, OP.min)
                    th0 = j1('th0')
                    V.tensor_tensor(out=th0[:], in0=hsc[:], in1=rapq[:], op=OP.mult)
                    th = j1('th')
                    V.tensor_scalar(th[:], th0[:], -1e17, 1e17, OP.max, OP.min)
                    th2 = j1('th2')
                    V.tensor_tensor(out=th2[:], in0=th[:], in1=th[:], op=OP.mult)
                    rt = j1('rt')
                    nc.scalar.activation(rt[:], th2[:], AF.Sqrt, bias=1.0)
                    thneg = j1('thneg')
                    V.tensor_scalar(thneg[:], th[:], -1.0, None, OP.mult)
                    absth = j1('absth')
                    V.tensor_tensor(out=absth[:], in0=th[:], in1=thneg[:], op=OP.max)
                    den = j1('den')
                    V.tensor_tensor(out=den[:], in0=absth[:], in1=rt[:], op=OP.add)
                    ge = j1('ge')
                    V.tensor_scalar(ge[:], th[:], 0.0, None, OP.is_ge)
                    sgn = j1('sgn')
                    V.tensor_scalar(sgn[:], ge[:], 2.0, -1.0, OP.mult, OP.add)
                    rden = j1('rden')
                    V.reciprocal(rden[:], den[:])
                    t_ = j1('t_')
                    V.tensor_tensor(out=t_[:], in0=sgn[:], in1=rden[:], op=OP.mult)
                    t2 = j1('t2')
                    V.tensor_tensor(out=t2[:], in0=t_[:], in1=t_[:], op=OP.mult)
                    rt2 = j1('rt2')
                    nc.scalar.activation(rt2[:], t2[:], AF.Sqrt, bias=1.0)
                    cc = j1('cc')
                    V.reciprocal(cc[:], rt2[:])
                    ss = j1('ss')
                    V.tensor_tensor(out=ss[:], in0=t_[:], in1=cc[:], op=OP.mult)
                    tneg = j1('tneg')
                    V.tensor_scalar(tneg[:], t_[:], -1.0, None, OP.mult)
                    ssneg = j1('ssneg')
                    V.tensor_scalar(ssneg[:], ss[:], -1.0, None, OP.mult)
                    V.scalar_tensor_tensor(out=dp, in0=apq, scalar=tneg[:, 0:1],
                                           in1=dp, op0=OP.mult, op1=OP.add)
                    V.scalar_tensor_tensor(out=dq, in0=apq, scalar=t_[:, 0:1],
                                           in1=dq, op0=OP.mult, op1=OP.add)
                    V.memset(apq, 0.0)
                    r_ = 3 - p_ - q_
                    kp = OIDX[(min(p_, r_), max(p_, r_))]
                    kq = OIDX[(min(q_, r_), max(q_, r_))]
                    apr = a6[:, kp:kp + 1]
                    aqr = a6[:, kq:kq + 1]
                    x1 = j1('x1')
                    V.tensor_scalar(x1[:], apr, cc[:, 0:1], None, OP.mult)
                    x2 = j1('x2')
                    V.tensor_scalar(x2[:], apr, ss[:, 0:1], None, OP.mult)
                    V.scalar_tensor_tensor(out=apr, in0=aqr, scalar=ssneg[:, 0:1],
                                           in1=x1[:], op0=OP.mult, op1=OP.add)
                    V.scalar_tensor_tensor(out=aqr, in0=aqr, scalar=cc[:, 0:1],
                                           in1=x2[:], op0=OP.mult, op1=OP.add)
                    wp = w9[:, 3 * p_:3 * p_ + 3]
                    wqr = w9[:, 3 * q_:3 * q_ + 3]
                    y1 = j3('y1')
                    V.tensor_scalar(y1[:], wp, cc[:, 0:1], None, OP.mult)
                    y2 = j3('y2')
                    V.tensor_scalar(y2[:], wp, ss[:, 0:1], None, OP.mult)
                    V.scalar_tensor_tensor(out=wp, in0=wqr, scalar=ssneg[:, 0:1],
                                           in1=y1[:], op0=OP.mult, op1=OP.add)
                    V.scalar_tensor_tensor(out=wqr, in0=wqr, scalar=cc[:, 0:1],
                                           in1=y2[:], op0=OP.mult, op1=OP.add)

            # ascending eigenvalue sort (3-element network)
            for (ai, bi_) in [(0, 1), (0, 2), (1, 2)]:
                da = a6[:, ai:ai + 1]
                db = a6[:, bi_:bi_ + 1]
                cmp = jcb.tile([1, 1], I32, tag='j1_cmp', name='j1_cmp')
                V.tensor_tensor(out=cmp[:], in0=da, in1=db, op=OP.is_le)
                dlo = j1('dlo')
                V.tensor_tensor(out=dlo[:], in0=da, in1=db, op=OP.min)
                dhi = j1('dhi')
                V.tensor_tensor(out=dhi[:], in0=da, in1=db, op=OP.max)
                V.tensor_copy(da, dlo[:])
                V.tensor_copy(db, dhi[:])
                wa = w9[:, 3 * ai:3 * ai + 3]
                wb = w9[:, 3 * bi_:3 * bi_ + 3]
                wlo = j3('wlo')
                V.select(out=wlo[:], mask=cmp[:].to_broadcast([1, 3]),
                         on_true=wa, on_false=wb)
                whi = j3('whi')
                V.select(out=whi[:], mask=cmp[:].to_broadcast([1, 3]),
                         on_true=wb, on_false=wa)
                V.tensor_copy(wa, wlo[:])
                V.tensor_copy(wb, whi[:])

            # spread w9 (1,9) across partitions -> (3,3) via DRAM roundtrip
            nc.sync.dma_start(jac_scr[0:9].rearrange('(a n) -> a n', a=1), w9[:])
            vt_f = jcb.tile([3, 3], F32, tag='vt_f', name='vt_f')
            nc.sync.dma_start(vt_f[:], jac_scr[0:9].rearrange('(r c) -> r c', r=3))
            vt_sb = geo.tile([3, 3], F32, tag='vt_sb', name='vt_sb')
            nc.scalar.copy(vt_sb[:], vt_f[:])

            # F_all (3, 24), Xc_fm (3, N), hX for all frames -> SBUF (fp16)
            f_ps = pmm.tile([3, 24], F32, tag='mm_ps', name='mm_ps', space='PSUM')
            mm(f_ps[:], vt_sb[:], s_sb[:], True, True)
            fa_sb = geo.tile([3, 24], F32, tag='fa_sb', name='fa_sb')
            nc.scalar.copy(fa_sb[:], f_ps[:])

            xc_ps = pau.tile([3, N], F32, tag='au_ps', name='au_ps', space='PSUM')
            mm(xc_ps[:], ident[0:3, 0:3], x_fm[:], True, False)
            mm(xc_ps[:], cneg[:], mask_row[:], False, True)
            xc_fm = geo.tile([3, N], F32, tag='xc_fm', name='xc_fm')
            nc.scalar.copy(xc_fm[:], xc_ps[:])

            hx_all = geo.tile([3, NF * N], F16, tag='hx_all', name='hx_all')
            for o in range(NF):
                hx_ps = pmm.tile([3, N], F32, tag='mm_ps', name='mm_ps',
                                 space='PSUM')
                mm(hx_ps[:], fa_sb[:, 3 * o:3 * o + 3], xc_fm[:], True, True)
                copy_on(eng('hx', o), hx_all[:, o * N:(o + 1) * N], hx_ps[:])

            # ---------------- layers ----------------

            # u tiles, batched per chunk: per lane u0 raw, u1h/u2h (pre-scaled
            # +bias), u3 raw. d=1 blocks are written frame-reversed. Tags cycle
            # (bufs=2) so blk k+2 reuses blk k's buffer.
            u_t = {}

            for lam in range(2):
                # frame-invariant part of layer-0 q
                if lam == 0:
                    qe_ps = pmm.tile([P, N], F32, tag='mm_ps', name='mm_ps',
                                     space='PSUM')
                    mm(qe_ps[:], wq_sb[0][1][:], embT[0][:], True, False)
                    mm(qe_ps[:], wq_sb[0][2][:], embT[1][:], False, True)
                    q_emb = geo.tile([P, N], F16, tag='q_emb', name='q_emb')
                    copy_on(eng('embq'), q_emb[:], qe_ps[:])

                def attention(f):
                    q_ps = pmm.tile([P, N], F32, tag='mm_ps', name='mm_ps',
                                    space='PSUM')
                    if lam == 0:
                        mm(q_ps[:], wq_sb[0][0][:], hx_all[:, f * N:(f + 1) * N],
                           True, False)
                        mm(q_ps[:], ident_h[:], q_emb[:], False, True)
                    else:
                        mm(q_ps[:], wq_sb[1][0][:], h2[(0, 0)][:, f * N:(f + 1) * N],
                           True, False)
                        mm(q_ps[:], wq_sb[1][1][:], h2[(0, 1)][:, f * N:(f + 1) * N],
                           False, True)
                    q_sb = att.tile([P, N], F16, tag='q_sb', name='q_sb')
                    copy_on(eng('q'), q_sb[:], q_ps[:])

                    k_ps = pmm.tile([P, N], F32, tag='mm_ps', name='mm_ps',
                                    space='PSUM')
                    mm(k_ps[:], wk_sb[lam][:], q_sb[:], True, True)
                    k_sb = att.tile([P, N], F16, tag='k_sb', name='k_sb')
                    copy_on(eng('k'), k_sb[:], k_ps[:])

                    q_tm = att.tile([P, N], F16, tag='q_tm', name='q_tm')
                    tpr = pmm.tile([P, N], F16, tag='mm_ps', name='mm_ps',
                                   space='PSUM')
                    for m in range(4):
                        nc.tensor.transpose(tpr[:, m * P:(m + 1) * P],
                                            q_sb[:, m * P:(m + 1) * P],
                                            ident_h[:])
                    copy_on(eng('qtm'), q_tm[:], tpr[:])

                    z_ps = pzt.tile([1, N], F32, tag='zt_ps', name='zt_ps',
                                    space='PSUM')
                    t_ps = pzt.tile([P, N], F32, tag='zt_ps', name='zt_ps',
                                    space='PSUM')
                    for m in range(4):
                        s_ps = psc.tile([P, N], F32, tag='s_ps', name='s_ps',
                                        space='PSUM')
                        mm(s_ps[:], k_sb[:, m * P:(m + 1) * P], q_sb[:], True, True)
                        e_sb = att.tile([P, N], F16, tag='e_sb', name='e_sb')
                        nc.scalar.activation(e_sb[:], s_ps[:], AF.Exp, scale=SCALE)
                        mm(z_ps[:], ones_col[:], e_sb[:], m == 0, m == 3)
                        mm(t_ps[:], q_tm[:, m * P:(m + 1) * P], e_sb[:], m == 0,
                           m == 3)
                    recip = att.tile([1, N], F16, tag='recip', name='recip')
                    nc.vector.reciprocal(recip[:], z_ps[:])
                    rb_ps = pmm.tile([P, N], F32, tag='mm_ps', name='mm_ps',
                                     space='PSUM')
                    mm(rb_ps[:], ones_row[:], recip[:], True, True)
                    rb_sb = att.tile([P, N], F16, tag='rb_sb', name='rb_sb')
                    copy_on(eng('rb'), rb_sb[:], rb_ps[:])
                    tn_sb = att.tile([P, N], F16, tag='tn_sb', name='tn_sb')
                    eng('tn').tensor_tensor(out=tn_sb[:], in0=t_ps[:], in1=rb_sb[:],
                                            op=OP.mult)

                    a_ps = pau.tile([P, N], F32, tag='au_ps', name='au_ps',
                                    space='PSUM')
                    mm(a_ps[:], wv_sb[lam][:], tn_sb[:], True, False)
                    mm(a_ps[:], ident_h[:], q_sb[:], False, True)
                    a_sb = att.tile([P, N], F16, tag='a_sb', name='a_sb')
                    copy_on(eng('acp'), a_sb[:], a_ps[:])

                    # U matmuls -> batched per-lane tiles (d=1 frame-reversed)
                    for j in range(8):
                        u_ps = pau.tile([P, N], F32, tag='au_ps', name='au_ps',
                                        space='PSUM')
                        mm(u_ps[:], wu_sb[lam][:, j * P:(j + 1) * P], a_sb[:],
                           True, True)
                        d = j // 4
                        mt = j % 4
                        fo = f % FBLK
                        dst = u_t[(d, f'u{mt}', f // FBLK)][
                            :, fo * N:(fo + 1) * N]
                        if d == 1:
                            dst = dst[:, ::-1]
                        if mt in (1, 2):
                            e = eng(f'u{mt}')
                            bias = bfh(lam, d) if mt == 1 else brh(lam, d)
                            if e is nc.scalar:
                                e.activation(dst, u_ps[:], AF.Identity,
                                             bias=bias, scale=0.5)
                            else:
                                e.tensor_scalar(dst, u_ps[:], 0.5, bias,
                                                OP.mult, OP.add)
                        elif mt == 0:
                            # u0h = 0.5*u0 (x = (t-1)*u0h later)
                            e = eng('u0')
                            if e is nc.scalar:
                                e.activation(dst, u_ps[:], AF.Identity,
                                             scale=0.5)
                            else:
                                e.tensor_scalar(dst, u_ps[:], 0.5, None,
                                                OP.mult)
                        else:
                            copy_on(eng('u3'), dst, u_ps[:])

                def scan_group(blks):
                    """All direction lanes of a group of FBLK-frame chunks,
                    emitted op-interleaved so the serial chains overlap on each
                    in-order engine stream. Chunk starts are frame starts, so
                    each chunk scans independently (carry resets at frame
                    boundaries via the f-boundary memsets)."""
                    DS = [(d, b) for b in blks for d in (0, 1)]
                    u0 = {d: u_t[(d[0], 'u0', d[1])][:] for d in DS}
                    u1h = {d: u_t[(d[0], 'u1', d[1])][:] for d in DS}
                    u2h = {d: u_t[(d[0], 'u2', d[1])][:] for d in DS}
                    u3 = {d: u_t[(d[0], 'u3', d[1])][:] for d in DS}

                    def st(base, d):
                        return scn.tile([P, CW], F16, tag=f'{base}{d[0]}_{d[1] % 2}',
                                        name=f'{base}{d[0]}')

                    c_buf = {}
                    for d in DS:
                        c_buf[d] = sc2.tile([P, CW + 1], F16,
                                            tag=f'c_buf{d[0]}_{d[1] % 2}',
                                            name=f'c_buf{d[0]}')
                        nc.vector.memset(c_buf[d][:, 0:1], 0.0)

                    # pass 0: gates from c=0; t = tanh(u1h) (u1h = .5*u1+.5bf)
                    t0, f_t, x_t = {}, {}, {}
                    for d in DS:
                        t0[d] = st('t_t', d)
                        nc.scalar.activation(t0[d][:], u1h[d], AF.Tanh)
                    for d in DS:
                        f_t[d] = st('f_t', d)
                        eng('f').tensor_scalar(f_t[d][:], t0[d][:], 0.5, 0.5,
                                               OP.mult, OP.add)
                        nc.vector.memset(f_t[d][:, 0:CW:N], 0.0)
                        x_t[d] = st('xg_t', d)
                        eng('x').scalar_tensor_tensor(
                            out=x_t[d][:], in0=t0[d][:], scalar=1.0, in1=u0[d],
                            op0=OP.subtract, op1=OP.mult)
                    for d in DS:
                        nc.vector.tensor_tensor_scan(
                            out=c_buf[d][:, 1:CW + 1], data0=f_t[d][:],
                            data1=x_t[d][:], initial=0.0, op0=OP.mult,
                            op1=OP.subtract)

                    # pass 1 (final): gates from c_prev
                    cv, ag, t1, f1, x1 = {}, {}, {}, {}, {}
                    for d in DS:
                        cv[d] = st('fm_t', d)
                        eng('cv').tensor_scalar(cv[d][:], c_buf[d][:, 0:CW],
                                                vfh(lam, d[0]), None, OP.mult)
                        nc.vector.memset(cv[d][:, 0:CW:N], 0.0)
                        ag[d] = st('xg_t', d)
                        eng('ag').tensor_tensor(out=ag[d][:], in0=cv[d][:],
                                                in1=u1h[d], op=OP.add)
                    for d in DS:
                        t1[d] = st('t_t', d)
                        nc.scalar.activation(t1[d][:], ag[d][:], AF.Tanh)
                    for d in DS:
                        f1[d] = st('f_t', d)
                        eng('f').tensor_scalar(f1[d][:], t1[d][:], 0.5, 0.5,
                                               OP.mult, OP.add)
                        nc.vector.memset(f1[d][:, 0:CW:N], 0.0)
                        x1[d] = st('xg_t', d)
                        eng('x').scalar_tensor_tensor(
                            out=x1[d][:], in0=t1[d][:], scalar=1.0, in1=u0[d],
                            op0=OP.subtract, op1=OP.mult)
                    for d in DS:
                        nc.vector.tensor_tensor_scan(
                            out=c_buf[d][:, 1:CW + 1], data0=f1[d][:],
                            data1=x1[d][:], initial=0.0, op0=OP.mult,
                            op1=OP.subtract)

                    # r gate + output
                    cvr, ar, tr, D_t, r_t, m_t = {}, {}, {}, {}, {}, {}
                    for d in DS:
                        cvr[d] = st('fm_t', d)
                        eng('cv').tensor_scalar(cvr[d][:], c_buf[d][:, 0:CW],
                                                vrh(lam, d[0]), None, OP.mult)
                        nc.vector.memset(cvr[d][:, 0:CW:N], 0.0)
                        ar[d] = st('xg_t', d)
                        eng('ar').tensor_tensor(out=ar[d][:], in0=cvr[d][:],
                                                in1=u2h[d], op=OP.add)
                    for d in DS:
                        tr[d] = st('t_t', d)
                        nc.scalar.activation(tr[d][:], ar[d][:], AF.Tanh)
                    for d in DS:
                        D_t[d] = st('fm_t', d)
                        eng('D').tensor_tensor(out=D_t[d][:],
                                               in0=c_buf[d][:, 1:CW + 1],
                                               in1=u3[d], op=OP.subtract)
                        r_t[d] = st('f_t', d)
                        eng('r').tensor_scalar(r_t[d][:], tr[d][:], 0.5, 0.5,
                                               OP.mult, OP.add)
                        m_t[d] = st('xg_t', d)
                        eng('m').tensor_tensor(out=m_t[d][:], in0=r_t[d][:],
                                               in1=D_t[d][:], op=OP.mult)
                    for d in DS:
                        blk = d[1]
                        c0 = blk * CW
                        dst = h2[(lam, d[0])]
                        if d[0] == 0:
                            eng('hf').tensor_tensor(out=dst[:, c0:c0 + CW],
                                                    in0=m_t[d][:], in1=u3[d],
                                                    op=OP.add)
                        else:
                            dv = dst.rearrange('p (f l) -> p f l', f=NF)[
                                :, blk * FBLK:(blk + 1) * FBLK, ::-1]
                            eng('hf').tensor_tensor(
                                out=dv,
                                in0=m_t[d][:].rearrange('p (f l) -> p f l',
                                                        f=FBLK),
                                in1=u3[d].rearrange('p (f l) -> p f l', f=FBLK),
                                op=OP.add)

                GRP = 2  # chunks per scan group (4 chains in flight)
                for g in range(0, NBLK, GRP):
                    blks = list(range(g, min(g + GRP, NBLK)))
                    for blk in blks:
                        for d in range(2):
                            for nm in ('u0', 'u1', 'u2', 'u3'):
                                u_t[(d, nm, blk)] = ust.tile(
                                    [P, CW], F16, tag=f'{nm}_{d}_{blk % 2}',
                                    name=f'{nm}_{d}')
                        for f in range(blk * FBLK, (blk + 1) * FBLK):
                            attention(f)
                    scan_group(blks)

        def out_section():
            """Mean over frames + transpose + store for the most recent h2.
            Emitted at the TOP of the loop body (software pipelining), reading
            the previous runtime iteration's h2, so each iteration's tail ends
            at the last scan and PE/ACT get ready work at iteration start."""
            h2d, ident = ST['h2'], ST['ident']
            for t in range(2):
                hv = h2d[(1, t)]
                s1 = mnp.tile([P, 4 * N], F16, tag=f'ms1_{t}', name=f'ms1_{t}')
                eng('mean').tensor_tensor(out=s1[:], in0=hv[:, 0:4 * N],
                                          in1=hv[:, 4 * N:8 * N], op=OP.add)
                s2 = mnp.tile([P, 2 * N], F16, tag=f'ms2_{t}', name=f'ms2_{t}')
                eng('mean').tensor_tensor(out=s2[:], in0=s1[:, 0:2 * N],
                                          in1=s1[:, 2 * N:4 * N], op=OP.add)
                m_fm = mnp.tile([P, N], F32, tag=f'm_fm{t}', name=f'm_fm{t}')
                eng('mean').tensor_tensor(out=m_fm[:], in0=s2[:, 0:N],
                                          in1=s2[:, N:2 * N], op=OP.add)
                for c in range(4):
                    tp = pmm.tile([P, P], F32, tag='mm_ps', name='mm_ps',
                                  space='PSUM')
                    nc.tensor.transpose(tp[:], m_fm[:, c * P:(c + 1) * P],
                                        ident[:])
                    o_st = pre.tile([P, P], F32, tag='o_st', name='o_st')
                    nc.scalar.activation(o_st[:], tp[:], AF.Copy, scale=1.0 / NF)
                    nc.sync.dma_start(out_d[c * P:(c + 1) * P,
                                            t * P:(t + 1) * P], o_st[:])

        with nc.allow_low_precision(reason='fp16 pipeline, within 2e-2 tol'):
            ST = init_static()
            if n_iter == 1:
                body()
            elif UNROLL:
                for i in range(n_iter):
                    if i > 0:
                        out_section()
                    body()
            else:
                with tc.For_i(0, n_iter, 1):
                    out_section()
                    body()
            out_section()

    nc.compile()
    return nc


_CACHE = {}


def _get_nc(n_iter=1):
    if n_iter not in _CACHE:
        _CACHE[n_iter] = _build(n_iter)
    return _CACHE[n_iter]


def host_inputs(inputs):
    """Build the 8 per-core input maps (pure slicing/packing/indexing)."""
    ops = np.array([[i, j, k] for i in (-1, 1) for j in (-1, 1) for k in (-1, 1)],
                   np.float32)
    S = np.zeros((3, 24), np.float32)
    for o in range(8):
        S[:, 3 * o:3 * o + 3] = np.diag(ops[o])

    shared = {'sops': S}
    for lam in range(2):
        shared[f'wq{lam}'] = np.ascontiguousarray(inputs[f'Wq{lam}'], np.float16)
        shared[f'wk{lam}'] = np.ascontiguousarray(inputs[f'Wk{lam}'], np.float16)
        shared[f'wv{lam}'] = np.ascontiguousarray(inputs[f'Wv{lam}'], np.float16)
        shared[f'wu{lam}'] = np.ascontiguousarray(inputs[f'Wu{lam}'], np.float16)
        shared[f'gates{lam}'] = np.ascontiguousarray(np.concatenate(
            [inputs[f'vf{lam}'], inputs[f'vr{lam}'],
             inputs[f'bf{lam}'], inputs[f'br{lam}']]), np.float32)

    emb = np.asarray(inputs['emb'], np.float32)
    seqs = np.asarray(inputs['noisy_seqs'])
    seqs = np.where(seqs < 0, 82, seqs)

    in_maps = []
    for b in range(B):
        m = dict(shared)
        m['x_ca'] = np.ascontiguousarray(inputs['noisy_cords'][b, :, 1, :],
                                         np.float32)
        m['maskr'] = np.ascontiguousarray(
            np.asarray(inputs['mask'][b], np.float32).reshape(1, N))
        m['embt'] = np.ascontiguousarray(emb[seqs[b]].T, np.float16)
        in_maps.append(m)
    return in_maps


def kernel(**inputs):
    nc = _get_nc(1)
    in_maps = host_inputs(inputs)
    res = run_bass_kernel_spmd(nc, in_maps, core_ids=list(range(B)))
    return np.stack([res.results[b]['out'] for b in range(B)], axis=0)


# revision 35
# speedup vs baseline: 1.3231x; 1.0711x over previous
"""Trainium2 Bass kernel for nn_FAEncoder: frame-averaged SRU++ encoder.

Sharding: data-parallel over B (8 samples -> 8 cores). Each core processes its
sample's 8 sign-flip frames end to end; weights are replicated. No collectives.

v2 design notes (vs the fp32r baseline):
- fp16 on the whole matmul path (PSUM accumulation stays fp32). This unlocks
  the DVE 2x/4x perf modes for tensor_scalar / tensor_tensor / copies.
- Gates use tanh instead of sigmoid: sigmoid(x) = 0.5*tanh(x/2)+0.5. exp and
  tanh live in the same activation-function table, so the ACT engine stops
  thrashing table loads between softmax and SRU gates (~1.3us per reload).
- Fixed point runs 2 passes total (pass0 + 1 refinement); method error vs the
  exact recurrence is ~1e-4 on this data distribution.
- SRU scan sections operate on frame-batched tiles [P, FBLK*N]: per-frame
  carry reset is done by zeroing the scan multiplier f at frame-start columns
  (strided memset), and zeroing the c_prev contribution in gate args.
- Embedding rows are gathered host-side (pure indexing) and shipped
  pre-transposed; the frame-invariant part of layer-0 q is computed once.
- hX for all frames lives in SBUF ([3, NF*N]); no DRAM scratch roundtrip.
- Engine assignment of copies/elementwise ops is table-driven (ASSIGN) and was
  tuned against the instruction-cost timeline simulator.
"""
import os
import sys
from contextlib import ExitStack

import numpy as np

for _p in ('/opt/trn_rl_repo', '/root/.axon_site/_ro/trn_rl_repo'):
    if os.path.isdir(_p) and _p not in sys.path:
        sys.path.append(_p)

import concourse.bacc as bacc
import concourse.bass as bass
import concourse.mybir as mybir
from concourse.bass_utils import run_bass_kernel_spmd
from concourse.masks import make_identity
from concourse.tile import TileContext

F32 = mybir.dt.float32
F16 = mybir.dt.float16
I32 = mybir.dt.int32
AF = mybir.ActivationFunctionType
OP = mybir.AluOpType

B, N, H, D2, EMB, VOCAB = 8, 512, 256, 128, 256, 100
NF, ND = 8, 2
P = 128
SCALE = float(1.0 / np.sqrt(np.float32(D2)))
SWEEPS = 2    # cyclic Jacobi sweeps for the 3x3 eigh (2.5e-3 end-to-end)
FBLK = 2      # frames per batched scan op
NBLK = NF // FBLK
CW = FBLK * N  # batched op width
SCN_BUFS = 1  # buffers for scan temp tiles
UNROLL = False  # python-unroll the timing loop (for TimelineSim)

# Engine assignment per op-site: 'a' = scalar/ACT, 'v' = vector/DVE,
# 'p' = gpsimd/Pool. Tuned against TimelineSim.
# NOTE: Pool/GPSIMD cannot access PSUM -- any op reading PSUM must be 'a'/'v'.
ASSIGN = {
    'hx': 'av',       # rotation for the 8 hX copies (PSUM)
    'embq': 'a',      # q_emb copy (PSUM)
    'q': 'a',         # PSUM
    'k': 'v',         # PSUM
    'qtm': 'v',       # merged q-transpose copy (fp16 PSUM -> 2x mode)
    'rb': 'a',        # PSUM
    'tn': 'v',        # PSUM operand
    'acp': 'a',       # PSUM
    'u0': 'v',        # PSUM (ts scale 0.5 -> u0h)
    'u1': 'a',        # PSUM
    'u2': 'a',        # PSUM
    'u3': 'v',        # PSUM
    # scan section (per chunk, all SBUF-only)
    'f': 'v',
    'x': 'v',
    'cv': 'v',
    'ag': 'v',
    'ar': 'v',
    'D': 'v',
    'r': 'v',
    'm': 'v',
    'hf': 'v',
    'mean': 'v',
}


def _build(n_iter=1):
    nc = bacc.Bacc('TRN2', num_devices=B, debug=False)

    def eng(key, idx=0):
        c = ASSIGN[key]
        c = c[idx % len(c)]
        return {'a': nc.scalar, 'v': nc.vector, 'p': nc.gpsimd}[c]

    def copy_on(e, out, in_):
        if e is nc.scalar:
            e.copy(out, in_)
        else:
            e.tensor_copy(out, in_)

    def mm(out, lhsT, rhs, start, stop):
        nc.tensor.matmul(out, lhsT, rhs, start=start, stop=stop)

    dt = nc.dram_tensor
    x_ca = dt('x_ca', [N, 3], F32, kind='ExternalInput').ap()
    maskr = dt('maskr', [1, N], F32, kind='ExternalInput').ap()
    embt = dt('embt', [EMB, N], F16, kind='ExternalInput').ap()
    sops = dt('sops', [3, 24], F32, kind='ExternalInput').ap()
    wq = [dt('wq0', [3 + EMB, D2], F16, kind='ExternalInput').ap(),
          dt('wq1', [H, D2], F16, kind='ExternalInput').ap()]
    wk = [dt(f'wk{i}', [D2, D2], F16, kind='ExternalInput').ap() for i in range(2)]
    wv = [dt(f'wv{i}', [D2, D2], F16, kind='ExternalInput').ap() for i in range(2)]
    wu = [dt(f'wu{i}', [D2, 8 * D2], F16, kind='ExternalInput').ap() for i in range(2)]
    gates = [dt(f'gates{i}', [8, P], F32, kind='ExternalInput').ap() for i in range(2)]
    out_d = dt('out', [N, H], F32, kind='ExternalOutput').ap()

    jac_scr = dt('jac_scr', [16], F32, kind='Internal').ap()

    with TileContext(nc) as tc, ExitStack() as es:
        cst = es.enter_context(tc.tile_pool(name='cst', bufs=1))
        geo = es.enter_context(tc.tile_pool(name='geo', bufs=2))
        pre = es.enter_context(tc.tile_pool(name='pre', bufs=2))
        mnp = es.enter_context(tc.tile_pool(name='mnp', bufs=1))
        att = es.enter_context(tc.tile_pool(name='att', bufs=2))
        ust = es.enter_context(tc.tile_pool(name='ust', bufs=1))
        scn = es.enter_context(tc.tile_pool(name='scn', bufs=SCN_BUFS))
        sc2 = es.enter_context(tc.tile_pool(name='sc2', bufs=1))
        h2p = es.enter_context(tc.tile_pool(name='h2p', bufs=1))
        h2q = es.enter_context(tc.tile_pool(name='h2q', bufs=1))
        jcb = es.enter_context(tc.tile_pool(name='jcb', bufs=2))
        pmm = es.enter_context(tc.tile_pool(name='pmm', bufs=2, space='PSUM'))
        psc = es.enter_context(tc.tile_pool(name='psc', bufs=2, space='PSUM'))
        pzt = es.enter_context(tc.tile_pool(name='pzt', bufs=2, space='PSUM'))
        pau = es.enter_context(tc.tile_pool(name='pau', bufs=2, space='PSUM'))

        def init_static():
            # one-time constants + persistent tiles (emitted once, pre-loop)
            st = {}
            ident = cst.tile([P, P], F32, tag='ident', name='ident')
            make_identity(nc, ident)
            ident_h = cst.tile([P, P], F16, tag='ident_h', name='ident_h')
            nc.scalar.copy(ident_h[:], ident[:])
            ones_col = cst.tile([P, 1], F16, tag='ones_col', name='ones_col')
            nc.vector.memset(ones_col[:], 1.0)
            ones_row = cst.tile([1, P], F16, tag='ones_row', name='ones_row')
            nc.vector.memset(ones_row[:], 1.0)
            ones_col_f = cst.tile([P, 1], F32, tag='ones_col_f',
                                  name='ones_col_f')
            nc.vector.memset(ones_col_f[:], 1.0)
            st['ident'], st['ident_h'] = ident, ident_h
            st['ones_col'], st['ones_row'] = ones_col, ones_row
            st['ones_col_f'] = ones_col_f
            h2 = {}
            for lam in range(2):
                for d in range(2):
                    pool = h2q if lam == 1 else h2p
                    h2[(lam, d)] = pool.tile([P, NF * N], F16,
                                             tag=f'h2_{lam}_{d}',
                                             name=f'h2_{lam}_{d}')
            st['h2'] = h2
            return st

        def body():
            ident = ST['ident']
            ident_h = ST['ident_h']
            ones_col = ST['ones_col']
            ones_row = ST['ones_row']
            ones_col_f = ST['ones_col_f']
            h2 = ST['h2']
            # ---------------- geometry inputs first (gate the serial eigh) ---
            x_tm = geo.tile([P, 12], F32, tag='x_tm', name='x_tm')
            nc.sync.dma_start(x_tm[:].rearrange('p (c i) -> p c i', c=4),
                              x_ca.rearrange('(c p) i -> p c i', p=P))
            x_fm = geo.tile([3, N], F32, tag='x_fm', name='x_fm')
            nc.sync.dma_start(x_fm[:], x_ca.rearrange('n i -> i n'))
            mask_col = geo.tile([P, 4], F32, tag='mask_col', name='mask_col')
            nc.sync.dma_start(mask_col[:], maskr[0].rearrange('(c p) -> p c', p=P))
            mask_row = cst.tile([1, N], F32, tag='mask_row', name='mask_row')
            nc.sync.dma_start(mask_row[:], maskr)
            s_sb = cst.tile([3, 24], F32, tag='s_sb', name='s_sb')
            nc.sync.dma_start(s_sb[:], sops)

            # ---------------- weight staging ----------------
            embT = [cst.tile([P, N], F16, tag=f'embT{t}', name=f'embT{t}')
                    for t in range(2)]
            for t in range(2):
                nc.sync.dma_start(embT[t][:], embt[t * P:(t + 1) * P, :])

            wq_sb = [[cst.tile([3, D2], F16, tag='wq0a', name='wq0a'),
                      cst.tile([P, D2], F16, tag='wq0b', name='wq0b'),
                      cst.tile([P, D2], F16, tag='wq0c', name='wq0c')],
                     [cst.tile([P, D2], F16, tag='wq1a', name='wq1a'),
                      cst.tile([P, D2], F16, tag='wq1b', name='wq1b')]]
            nc.sync.dma_start(wq_sb[0][0][:], wq[0][0:3, :])
            nc.sync.dma_start(wq_sb[0][1][:], wq[0][3:131, :])
            nc.sync.dma_start(wq_sb[0][2][:], wq[0][131:259, :])
            nc.sync.dma_start(wq_sb[1][0][:], wq[1][0:128, :])
            nc.sync.dma_start(wq_sb[1][1][:], wq[1][128:256, :])

            wk_sb, wv_sb, wu_sb, gt_sb, gth_sb = [], [], [], [], []
            for lam in range(2):
                wk_sb.append(cst.tile([D2, D2], F16, tag=f'wk{lam}', name=f'wk{lam}'))
                nc.sync.dma_start(wk_sb[lam][:], wk[lam])
                wv_sb.append(cst.tile([D2, D2], F16, tag=f'wv{lam}', name=f'wv{lam}'))
                nc.sync.dma_start(wv_sb[lam][:], wv[lam])
                wu_sb.append(cst.tile([D2, 8 * D2], F16, tag=f'wu{lam}',
                                      name=f'wu{lam}'))
                nc.sync.dma_start(wu_sb[lam][:], wu[lam])
                gt = cst.tile([P, 8], F32, tag=f'gt{lam}', name=f'gt{lam}')
                nc.sync.dma_start(gt[:], gates[lam].rearrange('g d -> d g'))
                gth = cst.tile([P, 8], F32, tag=f'gth{lam}', name=f'gth{lam}')
                nc.vector.tensor_scalar(gth[:], gt[:], 0.5, None, OP.mult)
                gt_sb.append(gt)
                gth_sb.append(gth)

            def vfh(lam, d):
                return gth_sb[lam][:, d:d + 1]

            def vrh(lam, d):
                return gth_sb[lam][:, 2 + d:3 + d]

            def bfh(lam, d):
                return gth_sb[lam][:, 4 + d:5 + d]

            def brh(lam, d):
                return gth_sb[lam][:, 6 + d:7 + d]

            # ---------------- frames: center, covariance, Jacobi eigh ----------------

            msum_ps = pzt.tile([1, 1], F32, tag='zt_ps', name='zt_ps', space='PSUM')
            mx_ps = pzt.tile([1, 3], F32, tag='zt_ps', name='zt_ps', space='PSUM')
            for c in range(4):
                mm(msum_ps[:], mask_col[:, c:c + 1], ones_col_f[:], c == 0, c == 3)
                mm(mx_ps[:], mask_col[:, c:c + 1], x_tm[:, 3 * c:3 * c + 3],
                   c == 0, c == 3)
            rmsum = jcb.tile([1, 1], F32, tag='rmsum', name='rmsum')
            nc.vector.reciprocal(rmsum[:], msum_ps[:])
            mx_sb = jcb.tile([1, 3], F32, tag='mx_sb', name='mx_sb')
            nc.scalar.copy(mx_sb[:], mx_ps[:])
            cneg = geo.tile([1, 3], F32, tag='cneg', name='cneg')
            nc.vector.tensor_scalar(cneg[:], mx_sb[:], rmsum[:, 0:1], -1.0,
                                    OP.mult, OP.mult)
            cneg_m = jcb.tile([1, 3], F32, tag='cneg_m', name='cneg_m')
            nc.vector.tensor_scalar(cneg_m[:], cneg[:], msum_ps[:, 0:1], None,
                                    OP.mult)

            c_ps = pmm.tile([3, 3], F32, tag='mm_ps', name='mm_ps', space='PSUM')
            for c in range(4):
                mm(c_ps[:], x_tm[:, 3 * c:3 * c + 3], x_tm[:, 3 * c:3 * c + 3],
                   c == 0, False)
            mm(c_ps[:], cneg[:], mx_sb[:], False, False)
            mm(c_ps[:], mx_sb[:], cneg[:], False, False)
            mm(c_ps[:], cneg[:], cneg_m[:], False, True)

            # Jacobi on flat layouts: a6 = [d0,d1,d2,o01,o02,o12], w9 = V^T rows
            c_sb = jcb.tile([3, 3], F32, tag='c_sb', name='c_sb')
            nc.scalar.copy(c_sb[:], c_ps[:])
            crow = []
            for r in range(3):
                row_ps = pmm.tile([1, 3], F32, tag='mm_ps', name='mm_ps',
                                  space='PSUM')
                mm(row_ps[:], ident[0:3, r:r + 1], c_sb[:], True, True)
                cr = jcb.tile([1, 3], F32, tag=f'crow{r}', name=f'crow{r}')
                nc.scalar.copy(cr[:], row_ps[:])
                crow.append(cr)
            a6 = jcb.tile([1, 6], F32, tag='a6', name='a6')
            for (k, (r_, c_)) in enumerate([(0, 0), (1, 1), (2, 2), (0, 1), (0, 2),
                                            (1, 2)]):
                nc.scalar.copy(a6[:, k:k + 1], crow[r_][:, c_:c_ + 1])
            w9 = jcb.tile([1, 9], F32, tag='w9', name='w9')
            nc.vector.memset(w9[:], 0.0)
            for i in range(3):
                nc.vector.memset(w9[:, 4 * i:4 * i + 1], 1.0)

            OIDX = {(0, 1): 3, (0, 2): 4, (1, 2): 5}
            V = nc.vector

            def j1(name):
                return jcb.tile([1, 1], F32, tag=f'j1_{name}', name=f'j1_{name}')

            def j3(name):
                return jcb.tile([1, 3], F32, tag=f'j3_{name}', name=f'j3_{name}')

            for _s in range(SWEEPS):
                for (p_, q_) in [(0, 1), (0, 2), (1, 2)]:
                    apq = a6[:, OIDX[(p_, q_)]:OIDX[(p_, q_)] + 1]
                    dp = a6[:, p_:p_ + 1]
                    dq = a6[:, q_:q_ + 1]
                    half = j1('half')
                    V.tensor_tensor(out=half[:], in0=dq, in1=dp, op=OP.subtract)
                    hsc = j1('hsc')
                    V.tensor_scalar(hsc[:], half[:], 0.5, None, OP.mult)
                    rapq0 = j1('rapq0')
                    V.reciprocal(rapq0[:], apq)
                    rapq = j1('rapq')
                    V.tensor_scalar(rapq[:], rapq0[:], -1e30, 1e30, OP.max, OP.min)
                    th0 = j1('th0')
                    V.tensor_tensor(out=th0[:], in0=hsc[:], in1=rapq[:], op=OP.mult)
                    th = j1('th')
                    V.tensor_scalar(th[:], th0[:], -1e17, 1e17, OP.max, OP.min)
                    th2 = j1('th2')
                    V.tensor_tensor(out=th2[:], in0=th[:], in1=th[:], op=OP.mult)
                    rt = j1('rt')
                    nc.scalar.activation(rt[:], th2[:], AF.Sqrt, bias=1.0)
                    thneg = j1('thneg')
                    V.tensor_scalar(thneg[:], th[:], -1.0, None, OP.mult)
                    absth = j1('absth')
                    V.tensor_tensor(out=absth[:], in0=th[:], in1=thneg[:], op=OP.max)
                    den = j1('den')
                    V.tensor_tensor(out=den[:], in0=absth[:], in1=rt[:], op=OP.add)
                    ge = j1('ge')
                    V.tensor_scalar(ge[:], th[:], 0.0, None, OP.is_ge)
                    sgn = j1('sgn')
                    V.tensor_scalar(sgn[:], ge[:], 2.0, -1.0, OP.mult, OP.add)
                    rden = j1('rden')
                    V.reciprocal(rden[:], den[:])
                    t_ = j1('t_')
                    V.tensor_tensor(out=t_[:], in0=sgn[:], in1=rden[:], op=OP.mult)
                    t2 = j1('t2')
                    V.tensor_tensor(out=t2[:], in0=t_[:], in1=t_[:], op=OP.mult)
                    rt2 = j1('rt2')
                    nc.scalar.activation(rt2[:], t2[:], AF.Sqrt, bias=1.0)
                    cc = j1('cc')
                    V.reciprocal(cc[:], rt2[:])
                    ss = j1('ss')
                    V.tensor_tensor(out=ss[:], in0=t_[:], in1=cc[:], op=OP.mult)
                    tneg = j1('tneg')
                    V.tensor_scalar(tneg[:], t_[:], -1.0, None, OP.mult)
                    ssneg = j1('ssneg')
                    V.tensor_scalar(ssneg[:], ss[:], -1.0, None, OP.mult)
                    V.scalar_tensor_tensor(out=dp, in0=apq, scalar=tneg[:, 0:1],
                                           in1=dp, op0=OP.mult, op1=OP.add)
                    V.scalar_tensor_tensor(out=dq, in0=apq, scalar=t_[:, 0:1],
                                           in1=dq, op0=OP.mult, op1=OP.add)
                    V.memset(apq, 0.0)
                    r_ = 3 - p_ - q_
                    kp = OIDX[(min(p_, r_), max(p_, r_))]
                    kq = OIDX[(min(q_, r_), max(q_, r_))]
                    apr = a6[:, kp:kp + 1]
                    aqr = a6[:, kq:kq + 1]
                    x1 = j1('x1')
                    V.tensor_scalar(x1[:], apr, cc[:, 0:1], None, OP.mult)
                    x2 = j1('x2')
                    V.tensor_scalar(x2[:], apr, ss[:, 0:1], None, OP.mult)
                    V.scalar_tensor_tensor(out=apr, in0=aqr, scalar=ssneg[:, 0:1],
                                           in1=x1[:], op0=OP.mult, op1=OP.add)
                    V.scalar_tensor_tensor(out=aqr, in0=aqr, scalar=cc[:, 0:1],
                                           in1=x2[:], op0=OP.mult, op1=OP.add)
                    wp = w9[:, 3 * p_:3 * p_ + 3]
                    wqr = w9[:, 3 * q_:3 * q_ + 3]
                    y1 = j3('y1')
                    V.tensor_scalar(y1[:], wp, cc[:, 0:1], None, OP.mult)
                    y2 = j3('y2')
                    V.tensor_scalar(y2[:], wp, ss[:, 0:1], None, OP.mult)
                    V.scalar_tensor_tensor(out=wp, in0=wqr, scalar=ssneg[:, 0:1],
                                           in1=y1[:], op0=OP.mult, op1=OP.add)
                    V.scalar_tensor_tensor(out=wqr, in0=wqr, scalar=cc[:, 0:1],
                                           in1=y2[:], op0=OP.mult, op1=OP.add)

            # ascending eigenvalue sort (3-element network)
            for (ai, bi_) in [(0, 1), (0, 2), (1, 2)]:
                da = a6[:, ai:ai + 1]
                db = a6[:, bi_:bi_ + 1]
                cmp = jcb.tile([1, 1], I32, tag='j1_cmp', name='j1_cmp')
                V.tensor_tensor(out=cmp[:], in0=da, in1=db, op=OP.is_le)
                dlo = j1('dlo')
                V.tensor_tensor(out=dlo[:], in0=da, in1=db, op=OP.min)
                dhi = j1('dhi')
                V.tensor_tensor(out=dhi[:], in0=da, in1=db, op=OP.max)
                V.tensor_copy(da, dlo[:])
                V.tensor_copy(db, dhi[:])
                wa = w9[:, 3 * ai:3 * ai + 3]
                wb = w9[:, 3 * bi_:3 * bi_ + 3]
                wlo = j3('wlo')
                V.select(out=wlo[:], mask=cmp[:].to_broadcast([1, 3]),
                         on_true=wa, on_false=wb)
                whi = j3('whi')
                V.select(out=whi[:], mask=cmp[:].to_broadcast([1, 3]),
                         on_true=wb, on_false=wa)
                V.tensor_copy(wa, wlo[:])
                V.tensor_copy(wb, whi[:])

            # spread w9 (1,9) across partitions -> (3,3) via DRAM roundtrip
            nc.sync.dma_start(jac_scr[0:9].rearrange('(a n) -> a n', a=1), w9[:])
            vt_f = jcb.tile([3, 3], F32, tag='vt_f', name='vt_f')
            nc.sync.dma_start(vt_f[:], jac_scr[0:9].rearrange('(r c) -> r c', r=3))
            vt_sb = geo.tile([3, 3], F32, tag='vt_sb', name='vt_sb')
            nc.scalar.copy(vt_sb[:], vt_f[:])

            # F_all (3, 24), Xc_fm (3, N), hX for all frames -> SBUF (fp16)
            f_ps = pmm.tile([3, 24], F32, tag='mm_ps', name='mm_ps', space='PSUM')
            mm(f_ps[:], vt_sb[:], s_sb[:], True, True)
            fa_sb = geo.tile([3, 24], F32, tag='fa_sb', name='fa_sb')
            nc.scalar.copy(fa_sb[:], f_ps[:])

            xc_ps = pau.tile([3, N], F32, tag='au_ps', name='au_ps', space='PSUM')
            mm(xc_ps[:], ident[0:3, 0:3], x_fm[:], True, False)
            mm(xc_ps[:], cneg[:], mask_row[:], False, True)
            xc_fm = geo.tile([3, N], F32, tag='xc_fm', name='xc_fm')
            nc.scalar.copy(xc_fm[:], xc_ps[:])

            hx_all = geo.tile([3, NF * N], F16, tag='hx_all', name='hx_all')
            for o in range(NF):
                hx_ps = pmm.tile([3, N], F32, tag='mm_ps', name='mm_ps',
                                 space='PSUM')
                mm(hx_ps[:], fa_sb[:, 3 * o:3 * o + 3], xc_fm[:], True, True)
                copy_on(eng('hx', o), hx_all[:, o * N:(o + 1) * N], hx_ps[:])

            # ---------------- layers ----------------

            # u tiles, batched per chunk: per lane u0 raw, u1h/u2h (pre-scaled
            # +bias), u3 raw. d=1 blocks are written frame-reversed. Tags cycle
            # (bufs=2) so blk k+2 reuses blk k's buffer.
            u_t = {}

            for lam in range(2):
                # frame-invariant part of layer-0 q
                if lam == 0:
                    qe_ps = pmm.tile([P, N], F32, tag='mm_ps', name='mm_ps',
                                     space='PSUM')
                    mm(qe_ps[:], wq_sb[0][1][:], embT[0][:], True, False)
                    mm(qe_ps[:], wq_sb[0][2][:], embT[1][:], False, True)
                    q_emb = geo.tile([P, N], F16, tag='q_emb', name='q_emb')
                    copy_on(eng('embq'), q_emb[:], qe_ps[:])

                def attention(f):
                    q_ps = pmm.tile([P, N], F32, tag='mm_ps', name='mm_ps',
                                    space='PSUM')
                    if lam == 0:
                        mm(q_ps[:], wq_sb[0][0][:], hx_all[:, f * N:(f + 1) * N],
                           True, False)
                        mm(q_ps[:], ident_h[:], q_emb[:], False, True)
                    else:
                        mm(q_ps[:], wq_sb[1][0][:], h2[(0, 0)][:, f * N:(f + 1) * N],
                           True, False)
                        mm(q_ps[:], wq_sb[1][1][:], h2[(0, 1)][:, f * N:(f + 1) * N],
                           False, True)
                    q_sb = att.tile([P, N], F16, tag='q_sb', name='q_sb')
                    copy_on(eng('q'), q_sb[:], q_ps[:])

                    k_ps = pmm.tile([P, N], F32, tag='mm_ps', name='mm_ps',
                                    space='PSUM')
                    mm(k_ps[:], wk_sb[lam][:], q_sb[:], True, True)
                    k_sb = att.tile([P, N], F16, tag='k_sb', name='k_sb')
                    copy_on(eng('k'), k_sb[:], k_ps[:])

                    q_tm = att.tile([P, N], F16, tag='q_tm', name='q_tm')
                    tpr = pmm.tile([P, N], F16, tag='mm_ps', name='mm_ps',
                                   space='PSUM')
                    for m in range(4):
                        nc.tensor.transpose(tpr[:, m * P:(m + 1) * P],
                                            q_sb[:, m * P:(m + 1) * P],
                                            ident_h[:])
                    copy_on(eng('qtm'), q_tm[:], tpr[:])

                    z_ps = pzt.tile([1, N], F32, tag='zt_ps', name='zt_ps',
                                    space='PSUM')
                    t_ps = pzt.tile([P, N], F32, tag='zt_ps', name='zt_ps',
                                    space='PSUM')
                    for m in range(4):
                        s_ps = psc.tile([P, N], F32, tag='s_ps', name='s_ps',
                                        space='PSUM')
                        mm(s_ps[:], k_sb[:, m * P:(m + 1) * P], q_sb[:], True, True)
                        e_sb = att.tile([P, N], F16, tag='e_sb', name='e_sb')
                        nc.scalar.activation(e_sb[:], s_ps[:], AF.Exp, scale=SCALE)
                        mm(z_ps[:], ones_col[:], e_sb[:], m == 0, m == 3)
                        mm(t_ps[:], q_tm[:, m * P:(m + 1) * P], e_sb[:], m == 0,
                           m == 3)
                    recip = att.tile([1, N], F16, tag='recip', name='recip')
                    nc.vector.reciprocal(recip[:], z_ps[:])
                    rb_ps = pmm.tile([P, N], F32, tag='mm_ps', name='mm_ps',
                                     space='PSUM')
                    mm(rb_ps[:], ones_row[:], recip[:], True, True)
                    rb_sb = att.tile([P, N], F16, tag='rb_sb', name='rb_sb')
                    copy_on(eng('rb'), rb_sb[:], rb_ps[:])
                    tn_sb = att.tile([P, N], F16, tag='tn_sb', name='tn_sb')
                    eng('tn').tensor_tensor(out=tn_sb[:], in0=t_ps[:], in1=rb_sb[:],
                                            op=OP.mult)

                    a_ps = pau.tile([P, N], F32, tag='au_ps', name='au_ps',
                                    space='PSUM')
                    mm(a_ps[:], wv_sb[lam][:], tn_sb[:], True, False)
                    mm(a_ps[:], ident_h[:], q_sb[:], False, True)
                    a_sb = att.tile([P, N], F16, tag='a_sb', name='a_sb')
                    copy_on(eng('acp'), a_sb[:], a_ps[:])

                    # U matmuls -> batched per-lane tiles (d=1 frame-reversed)
                    for j in range(8):
                        u_ps = pau.tile([P, N], F32, tag='au_ps', name='au_ps',
                                        space='PSUM')
                        mm(u_ps[:], wu_sb[lam][:, j * P:(j + 1) * P], a_sb[:],
                           True, True)
                        d = j // 4
                        mt = j % 4
                        fo = f % FBLK
                        dst = u_t[(d, f'u{mt}', f // FBLK)][
                            :, fo * N:(fo + 1) * N]
                        if d == 1:
                            dst = dst[:, ::-1]
                        if mt in (1, 2):
                            e = eng(f'u{mt}')
                            bias = bfh(lam, d) if mt == 1 else brh(lam, d)
                            if e is nc.scalar:
                                e.activation(dst, u_ps[:], AF.Identity,
                                             bias=bias, scale=0.5)
                            else:
                                e.tensor_scalar(dst, u_ps[:], 0.5, bias,
                                                OP.mult, OP.add)
                        elif mt == 0:
                            # u0h = 0.5*u0 (x = (t-1)*u0h later)
                            e = eng('u0')
                            if e is nc.scalar:
                                e.activation(dst, u_ps[:], AF.Identity,
                                             scale=0.5)
                            else:
                                e.tensor_scalar(dst, u_ps[:], 0.5, None,
                                                OP.mult)
                        else:
                            copy_on(eng('u3'), dst, u_ps[:])

                def scan_group(blks):
                    """All direction lanes of a group of FBLK-frame chunks,
                    emitted op-interleaved so the serial chains overlap on each
                    in-order engine stream. Chunk starts are frame starts, so
                    each chunk scans independently (carry resets at frame
                    boundaries via the f-boundary memsets)."""
                    DS = [(d, b) for b in blks for d in (0, 1)]
                    u0 = {d: u_t[(d[0], 'u0', d[1])][:] for d in DS}
                    u1h = {d: u_t[(d[0], 'u1', d[1])][:] for d in DS}
                    u2h = {d: u_t[(d[0], 'u2', d[1])][:] for d in DS}
                    u3 = {d: u_t[(d[0], 'u3', d[1])][:] for d in DS}

                    def st(base, d):
                        return scn.tile([P, CW], F16, tag=f'{base}{d[0]}_{d[1] % 2}',
                                        name=f'{base}{d[0]}')

                    c_buf = {}
                    for d in DS:
                        c_buf[d] = sc2.tile([P, CW + 1], F16,
                                            tag=f'c_buf{d[0]}_{d[1] % 2}',
                                            name=f'c_buf{d[0]}')
                        nc.vector.memset(c_buf[d][:, 0:1], 0.0)

                    # pass 0: gates from c=0; t = tanh(u1h) (u1h = .5*u1+.5bf)
                    t0, f_t, x_t = {}, {}, {}
                    for d in DS:
                        t0[d] = st('t_t', d)
                        nc.scalar.activation(t0[d][:], u1h[d], AF.Tanh)
                    for d in DS:
                        f_t[d] = st('f_t', d)
                        eng('f').tensor_scalar(f_t[d][:], t0[d][:], 0.5, 0.5,
                                               OP.mult, OP.add)
                        nc.vector.memset(f_t[d][:, 0:CW:N], 0.0)
                        x_t[d] = st('xg_t', d)
                        eng('x').scalar_tensor_tensor(
                            out=x_t[d][:], in0=t0[d][:], scalar=1.0, in1=u0[d],
                            op0=OP.subtract, op1=OP.mult)
                    for d in DS:
                        nc.vector.tensor_tensor_scan(
                            out=c_buf[d][:, 1:CW + 1], data0=f_t[d][:],
                            data1=x_t[d][:], initial=0.0, op0=OP.mult,
                            op1=OP.subtract)

                    # pass 1 (final): gates from c_prev
                    cv, ag, t1, f1, x1 = {}, {}, {}, {}, {}
                    for d in DS:
                        cv[d] = st('fm_t', d)
                        eng('cv').tensor_scalar(cv[d][:], c_buf[d][:, 0:CW],
                                                vfh(lam, d[0]), None, OP.mult)
                        nc.vector.memset(cv[d][:, 0:CW:N], 0.0)
                        ag[d] = st('xg_t', d)
                        eng('ag').tensor_tensor(out=ag[d][:], in0=cv[d][:],
                                                in1=u1h[d], op=OP.add)
                    for d in DS:
                        t1[d] = st('t_t', d)
                        nc.scalar.activation(t1[d][:], ag[d][:], AF.Tanh)
                    for d in DS:
                        f1[d] = st('f_t', d)
                        eng('f').tensor_scalar(f1[d][:], t1[d][:], 0.5, 0.5,
                                               OP.mult, OP.add)
                        nc.vector.memset(f1[d][:, 0:CW:N], 0.0)
                        x1[d] = st('xg_t', d)
                        eng('x').scalar_tensor_tensor(
                            out=x1[d][:], in0=t1[d][:], scalar=1.0, in1=u0[d],
                            op0=OP.subtract, op1=OP.mult)
                    for d in DS:
                        nc.vector.tensor_tensor_scan(
                            out=c_buf[d][:, 1:CW + 1], data0=f1[d][:],
                            data1=x1[d][:], initial=0.0, op0=OP.mult,
                            op1=OP.subtract)

                    # r gate + output
                    cvr, ar, tr, D_t, r_t, m_t = {}, {}, {}, {}, {}, {}
                    for d in DS:
                        cvr[d] = st('fm_t', d)
                        eng('cv').tensor_scalar(cvr[d][:], c_buf[d][:, 0:CW],
                                                vrh(lam, d[0]), None, OP.mult)
                        nc.vector.memset(cvr[d][:, 0:CW:N], 0.0)
                        ar[d] = st('xg_t', d)
                        eng('ar').tensor_tensor(out=ar[d][:], in0=cvr[d][:],
                                                in1=u2h[d], op=OP.add)
                    for d in DS:
                        tr[d] = st('t_t', d)
                        nc.scalar.activation(tr[d][:], ar[d][:], AF.Tanh)
                    for d in DS:
                        D_t[d] = st('fm_t', d)
                        eng('D').tensor_tensor(out=D_t[d][:],
                                               in0=c_buf[d][:, 1:CW + 1],
                                               in1=u3[d], op=OP.subtract)
                        r_t[d] = st('f_t', d)
                        eng('r').tensor_scalar(r_t[d][:], tr[d][:], 0.5, 0.5,
                                               OP.mult, OP.add)
                        m_t[d] = st('xg_t', d)
                        eng('m').tensor_tensor(out=m_t[d][:], in0=r_t[d][:],
                                               in1=D_t[d][:], op=OP.mult)
                    for d in DS:
                        blk = d[1]
                        c0 = blk * CW
                        dst = h2[(lam, d[0])]
                        if d[0] == 0:
                            eng('hf').tensor_tensor(out=dst[:, c0:c0 + CW],
                                                    in0=m_t[d][:], in1=u3[d],
                                                    op=OP.add)
                        else:
                            dv = dst.rearrange('p (f l) -> p f l', f=NF)[
                                :, blk * FBLK:(blk + 1) * FBLK, ::-1]
                            eng('hf').tensor_tensor(
                                out=dv,
                                in0=m_t[d][:].rearrange('p (f l) -> p f l',
                                                        f=FBLK),
                                in1=u3[d].rearrange('p (f l) -> p f l', f=FBLK),
                                op=OP.add)

                GRP = 2  # chunks per scan group (4 chains in flight)
                for g in range(0, NBLK, GRP):
                    blks = list(range(g, min(g + GRP, NBLK)))
                    for blk in blks:
                        for d in range(2):
                            for nm in ('u0', 'u1', 'u2', 'u3'):
                                u_t[(d, nm, blk)] = ust.tile(
                                    [P, CW], F16, tag=f'{nm}_{d}_{blk % 2}',
                                    name=f'{nm}_{d}')
                        for f in range(blk * FBLK, (blk + 1) * FBLK):
                            attention(f)
                    scan_group(blks)

        def out_section():
            """Mean over frames + transpose + store for the most recent h2.
            Emitted at the TOP of the loop body (software pipelining), reading
            the previous runtime iteration's h2, so each iteration's tail ends
            at the last scan and PE/ACT get ready work at iteration start."""
            h2d, ident = ST['h2'], ST['ident']
            for t in range(2):
                hv = h2d[(1, t)]
                s1 = mnp.tile([P, 4 * N], F16, tag=f'ms1_{t}', name=f'ms1_{t}')
                eng('mean').tensor_tensor(out=s1[:], in0=hv[:, 0:4 * N],
                                          in1=hv[:, 4 * N:8 * N], op=OP.add)
                s2 = mnp.tile([P, 2 * N], F16, tag=f'ms2_{t}', name=f'ms2_{t}')
                eng('mean').tensor_tensor(out=s2[:], in0=s1[:, 0:2 * N],
                                          in1=s1[:, 2 * N:4 * N], op=OP.add)
                m_fm = mnp.tile([P, N], F32, tag=f'm_fm{t}', name=f'm_fm{t}')
                eng('mean').tensor_tensor(out=m_fm[:], in0=s2[:, 0:N],
                                          in1=s2[:, N:2 * N], op=OP.add)
                for c in range(4):
                    tp = pmm.tile([P, P], F32, tag='mm_ps', name='mm_ps',
                                  space='PSUM')
                    nc.tensor.transpose(tp[:], m_fm[:, c * P:(c + 1) * P],
                                        ident[:])
                    o_st = pre.tile([P, P], F32, tag='o_st', name='o_st')
                    nc.scalar.activation(o_st[:], tp[:], AF.Copy, scale=1.0 / NF)
                    nc.sync.dma_start(out_d[c * P:(c + 1) * P,
                                            t * P:(t + 1) * P], o_st[:])

        with nc.allow_low_precision(reason='fp16 pipeline, within 2e-2 tol'):
            ST = init_static()
            if n_iter == 1:
                body()
            elif UNROLL:
                for i in range(n_iter):
                    if i > 0:
                        out_section()
                    body()
            else:
                with tc.For_i(0, n_iter, 1):
                    out_section()
                    body()
            out_section()

    nc.compile()
    return nc


_CACHE = {}


def _get_nc(n_iter=1):
    if n_iter not in _CACHE:
        _CACHE[n_iter] = _build(n_iter)
    return _CACHE[n_iter]


def host_inputs(inputs):
    """Build the 8 per-core input maps (pure slicing/packing/indexing)."""
    ops = np.array([[i, j, k] for i in (-1, 1) for j in (-1, 1) for k in (-1, 1)],
                   np.float32)
    S = np.zeros((3, 24), np.float32)
    for o in range(8):
        S[:, 3 * o:3 * o + 3] = np.diag(ops[o])

    shared = {'sops': S}
    for lam in range(2):
        shared[f'wq{lam}'] = np.ascontiguousarray(inputs[f'Wq{lam}'], np.float16)
        shared[f'wk{lam}'] = np.ascontiguousarray(inputs[f'Wk{lam}'], np.float16)
        shared[f'wv{lam}'] = np.ascontiguousarray(inputs[f'Wv{lam}'], np.float16)
        shared[f'wu{lam}'] = np.ascontiguousarray(inputs[f'Wu{lam}'], np.float16)
        shared[f'gates{lam}'] = np.ascontiguousarray(np.concatenate(
            [inputs[f'vf{lam}'], inputs[f'vr{lam}'],
             inputs[f'bf{lam}'], inputs[f'br{lam}']]), np.float32)

    emb = np.asarray(inputs['emb'], np.float32)
    seqs = np.asarray(inputs['noisy_seqs'])
    seqs = np.where(seqs < 0, 82, seqs)

    in_maps = []
    for b in range(B):
        m = dict(shared)
        m['x_ca'] = np.ascontiguousarray(inputs['noisy_cords'][b, :, 1, :],
                                         np.float32)
        m['maskr'] = np.ascontiguousarray(
            np.asarray(inputs['mask'][b], np.float32).reshape(1, N))
        m['embt'] = np.ascontiguousarray(emb[seqs[b]].T, np.float16)
        in_maps.append(m)
    return in_maps


def kernel(**inputs):
    nc = _get_nc(1)
    in_maps = host_inputs(inputs)
    res = run_bass_kernel_spmd(nc, in_maps, core_ids=list(range(B)))
    return np.stack([res.results[b]['out'] for b in range(B)], axis=0)


# revision 36
# speedup vs baseline: 1.5093x; 1.1407x over previous
"""Trainium2 Bass kernel for nn_FAEncoder: frame-averaged SRU++ encoder.

Sharding: data-parallel over B (8 samples -> 8 cores). Each core processes its
sample's 8 sign-flip frames end to end; weights are replicated. No collectives.

v2 design notes (vs the fp32r baseline):
- fp16 on the whole matmul path (PSUM accumulation stays fp32). This unlocks
  the DVE 2x/4x perf modes for tensor_scalar / tensor_tensor / copies.
- Gates use tanh instead of sigmoid: sigmoid(x) = 0.5*tanh(x/2)+0.5. exp and
  tanh live in the same activation-function table, so the ACT engine stops
  thrashing table loads between softmax and SRU gates (~1.3us per reload).
- Fixed point runs 2 passes total (pass0 + 1 refinement); method error vs the
  exact recurrence is ~1e-4 on this data distribution.
- SRU scan sections operate on frame-batched tiles [P, FBLK*N]: per-frame
  carry reset is done by zeroing the scan multiplier f at frame-start columns
  (strided memset), and zeroing the c_prev contribution in gate args.
- Embedding rows are gathered host-side (pure indexing) and shipped
  pre-transposed; the frame-invariant part of layer-0 q is computed once.
- hX for all frames lives in SBUF ([3, NF*N]); no DRAM scratch roundtrip.
- Engine assignment of copies/elementwise ops is table-driven (ASSIGN) and was
  tuned against the instruction-cost timeline simulator.
"""
import os
import sys
from contextlib import ExitStack

import numpy as np

for _p in ('/opt/trn_rl_repo', '/root/.axon_site/_ro/trn_rl_repo'):
    if os.path.isdir(_p) and _p not in sys.path:
        sys.path.append(_p)

import concourse.bacc as bacc
import concourse.bass as bass
import concourse.mybir as mybir
from concourse.bass_utils import run_bass_kernel_spmd
from concourse.masks import make_identity
from concourse.tile import TileContext

F32 = mybir.dt.float32
F16 = mybir.dt.float16
I32 = mybir.dt.int32
AF = mybir.ActivationFunctionType
OP = mybir.AluOpType

B, N, H, D2, EMB, VOCAB = 8, 512, 256, 128, 256, 100
NF, ND = 8, 2
P = 128
SCALE = float(1.0 / np.sqrt(np.float32(D2)))
SWEEPS = 2    # cyclic Jacobi sweeps for the 3x3 eigh (2.5e-3 end-to-end)
FBLK = 2      # frames per batched scan op
NBLK = NF // FBLK
CW = FBLK * N  # batched op width
SCN_BUFS = 1  # buffers for scan temp tiles
UNROLL = False  # python-unroll the timing loop (for TimelineSim)

# Engine assignment per op-site: 'a' = scalar/ACT, 'v' = vector/DVE,
# 'p' = gpsimd/Pool. Tuned against TimelineSim.
# NOTE: Pool/GPSIMD cannot access PSUM -- any op reading PSUM must be 'a'/'v'.
ASSIGN = {
    'hx': 'av',       # rotation for the 8 hX copies (PSUM)
    'embq': 'a',      # q_emb copy (PSUM)
    'q': 'a',         # PSUM
    'k': 'v',         # PSUM
    'qtm': 'v',       # merged q-transpose copy (fp16 PSUM -> 2x mode)
    'rb': 'a',        # PSUM
    'tn': 'v',        # PSUM operand
    'acp': 'a',       # PSUM
    'u0': 'v',        # PSUM (ts scale 0.5 -> u0h)
    'u1': 'a',        # PSUM
    'u2': 'a',        # PSUM
    'u3': 'v',        # PSUM
    # scan section (per chunk, all SBUF-only)
    'f': 'v',
    'x': 'v',
    'cv': 'v',
    'ag': 'v',
    'ar': 'v',
    'D': 'v',
    'r': 'v',
    'm': 'v',
    'hf': 'v',
    'mean': 'v',
}


def _build(n_iter=1):
    nc = bacc.Bacc('TRN2', num_devices=B, debug=False)

    def eng(key, idx=0):
        c = ASSIGN[key]
        c = c[idx % len(c)]
        return {'a': nc.scalar, 'v': nc.vector, 'p': nc.gpsimd}[c]

    def copy_on(e, out, in_):
        if e is nc.scalar:
            e.copy(out, in_)
        else:
            e.tensor_copy(out, in_)

    def mm(out, lhsT, rhs, start, stop):
        nc.tensor.matmul(out, lhsT, rhs, start=start, stop=stop)

    dt = nc.dram_tensor
    x_ca = dt('x_ca', [N, 3], F32, kind='ExternalInput').ap()
    maskr = dt('maskr', [1, N], F32, kind='ExternalInput').ap()
    embt = dt('embt', [EMB, N], F16, kind='ExternalInput').ap()
    sops = dt('sops', [3, 24], F32, kind='ExternalInput').ap()
    wq = [dt('wq0', [3 + EMB, D2], F16, kind='ExternalInput').ap(),
          dt('wq1', [H, D2], F16, kind='ExternalInput').ap()]
    wk = [dt(f'wk{i}', [D2, D2], F16, kind='ExternalInput').ap() for i in range(2)]
    wv = [dt(f'wv{i}', [D2, D2], F16, kind='ExternalInput').ap() for i in range(2)]
    wu = [dt(f'wu{i}', [D2, 8 * D2], F16, kind='ExternalInput').ap() for i in range(2)]
    gates = [dt(f'gates{i}', [8, P], F32, kind='ExternalInput').ap() for i in range(2)]
    out_d = dt('out', [N, H], F32, kind='ExternalOutput').ap()

    jac_scr = dt('jac_scr', [16], F32, kind='Internal').ap()

    with TileContext(nc) as tc, ExitStack() as es:
        cst = es.enter_context(tc.tile_pool(name='cst', bufs=1))
        geo = es.enter_context(tc.tile_pool(name='geo', bufs=2))
        pre = es.enter_context(tc.tile_pool(name='pre', bufs=2))
        mnp = es.enter_context(tc.tile_pool(name='mnp', bufs=1))
        att = es.enter_context(tc.tile_pool(name='att', bufs=2))
        ust = es.enter_context(tc.tile_pool(name='ust', bufs=1))
        scn = es.enter_context(tc.tile_pool(name='scn', bufs=SCN_BUFS))
        sc2 = es.enter_context(tc.tile_pool(name='sc2', bufs=1))
        h2p = es.enter_context(tc.tile_pool(name='h2p', bufs=1))
        h2q = es.enter_context(tc.tile_pool(name='h2q', bufs=1))
        jcb = es.enter_context(tc.tile_pool(name='jcb', bufs=2))
        pmm = es.enter_context(tc.tile_pool(name='pmm', bufs=2, space='PSUM'))
        psc = es.enter_context(tc.tile_pool(name='psc', bufs=2, space='PSUM'))
        pzt = es.enter_context(tc.tile_pool(name='pzt', bufs=2, space='PSUM'))
        pau = es.enter_context(tc.tile_pool(name='pau', bufs=2, space='PSUM'))

        def init_static():
            # one-time constants + persistent tiles (emitted once, pre-loop)
            st = {}
            ident = cst.tile([P, P], F32, tag='ident', name='ident')
            make_identity(nc, ident)
            ident_h = cst.tile([P, P], F16, tag='ident_h', name='ident_h')
            nc.scalar.copy(ident_h[:], ident[:])
            ones_col = cst.tile([P, 1], F16, tag='ones_col', name='ones_col')
            nc.vector.memset(ones_col[:], 1.0)
            ones_row = cst.tile([1, P], F16, tag='ones_row', name='ones_row')
            nc.vector.memset(ones_row[:], 1.0)
            ones_col_f = cst.tile([P, 1], F32, tag='ones_col_f',
                                  name='ones_col_f')
            nc.vector.memset(ones_col_f[:], 1.0)
            st['ident'], st['ident_h'] = ident, ident_h
            st['ones_col'], st['ones_row'] = ones_col, ones_row
            st['ones_col_f'] = ones_col_f
            h2 = {}
            for lam in range(2):
                for d in range(2):
                    pool = h2q if lam == 1 else h2p
                    h2[(lam, d)] = pool.tile([P, NF * N], F16,
                                             tag=f'h2_{lam}_{d}',
                                             name=f'h2_{lam}_{d}')
            st['h2'] = h2
            return st

        def body():
            ident = ST['ident']
            ident_h = ST['ident_h']
            ones_col = ST['ones_col']
            ones_row = ST['ones_row']
            ones_col_f = ST['ones_col_f']
            h2 = ST['h2']
            # ---------------- geometry inputs first (gate the serial eigh) ---
            x_tm = geo.tile([P, 12], F32, tag='x_tm', name='x_tm')
            nc.sync.dma_start(x_tm[:].rearrange('p (c i) -> p c i', c=4),
                              x_ca.rearrange('(c p) i -> p c i', p=P))
            x_fm = geo.tile([3, N], F32, tag='x_fm', name='x_fm')
            nc.sync.dma_start(x_fm[:], x_ca.rearrange('n i -> i n'))
            mask_col = geo.tile([P, 4], F32, tag='mask_col', name='mask_col')
            nc.sync.dma_start(mask_col[:], maskr[0].rearrange('(c p) -> p c', p=P))
            mask_row = cst.tile([1, N], F32, tag='mask_row', name='mask_row')
            nc.sync.dma_start(mask_row[:], maskr)
            s_sb = cst.tile([3, 24], F32, tag='s_sb', name='s_sb')
            nc.sync.dma_start(s_sb[:], sops)

            # ---------------- weight staging ----------------
            embT = [cst.tile([P, N], F16, tag=f'embT{t}', name=f'embT{t}')
                    for t in range(2)]
            for t in range(2):
                nc.sync.dma_start(embT[t][:], embt[t * P:(t + 1) * P, :])

            wq_sb = [[cst.tile([3, D2], F16, tag='wq0a', name='wq0a'),
                      cst.tile([P, D2], F16, tag='wq0b', name='wq0b'),
                      cst.tile([P, D2], F16, tag='wq0c', name='wq0c')],
                     [cst.tile([P, D2], F16, tag='wq1a', name='wq1a'),
                      cst.tile([P, D2], F16, tag='wq1b', name='wq1b')]]
            nc.sync.dma_start(wq_sb[0][0][:], wq[0][0:3, :])
            nc.sync.dma_start(wq_sb[0][1][:], wq[0][3:131, :])
            nc.sync.dma_start(wq_sb[0][2][:], wq[0][131:259, :])
            nc.sync.dma_start(wq_sb[1][0][:], wq[1][0:128, :])
            nc.sync.dma_start(wq_sb[1][1][:], wq[1][128:256, :])

            wk_sb, wv_sb, wu_sb, gt_sb, gth_sb = [], [], [], [], []
            for lam in range(2):
                wk_sb.append(cst.tile([D2, D2], F16, tag=f'wk{lam}', name=f'wk{lam}'))
                nc.sync.dma_start(wk_sb[lam][:], wk[lam])
                wv_sb.append(cst.tile([D2, D2], F16, tag=f'wv{lam}', name=f'wv{lam}'))
                nc.sync.dma_start(wv_sb[lam][:], wv[lam])
                wu_sb.append(cst.tile([D2, 8 * D2], F16, tag=f'wu{lam}',
                                      name=f'wu{lam}'))
                nc.sync.dma_start(wu_sb[lam][:], wu[lam])
                gt = cst.tile([P, 8], F32, tag=f'gt{lam}', name=f'gt{lam}')
                nc.sync.dma_start(gt[:], gates[lam].rearrange('g d -> d g'))
                gth = cst.tile([P, 8], F32, tag=f'gth{lam}', name=f'gth{lam}')
                nc.vector.tensor_scalar(gth[:], gt[:], 0.5, None, OP.mult)
                gt_sb.append(gt)
                gth_sb.append(gth)

            def vfh(lam, d):
                return gth_sb[lam][:, d:d + 1]

            def vrh(lam, d):
                return gth_sb[lam][:, 2 + d:3 + d]

            def bfh(lam, d):
                return gth_sb[lam][:, 4 + d:5 + d]

            def brh(lam, d):
                return gth_sb[lam][:, 6 + d:7 + d]

            # ---------------- frames: center, covariance, Jacobi eigh ----------------

            msum_ps = pzt.tile([1, 1], F32, tag='zt_ps', name='zt_ps', space='PSUM')
            mx_ps = pzt.tile([1, 3], F32, tag='zt_ps', name='zt_ps', space='PSUM')
            for c in range(4):
                mm(msum_ps[:], mask_col[:, c:c + 1], ones_col_f[:], c == 0, c == 3)
                mm(mx_ps[:], mask_col[:, c:c + 1], x_tm[:, 3 * c:3 * c + 3],
                   c == 0, c == 3)
            rmsum = jcb.tile([1, 1], F32, tag='rmsum', name='rmsum')
            nc.vector.reciprocal(rmsum[:], msum_ps[:])
            mx_sb = jcb.tile([1, 3], F32, tag='mx_sb', name='mx_sb')
            nc.scalar.copy(mx_sb[:], mx_ps[:])
            cneg = geo.tile([1, 3], F32, tag='cneg', name='cneg')
            nc.vector.tensor_scalar(cneg[:], mx_sb[:], rmsum[:, 0:1], -1.0,
                                    OP.mult, OP.mult)
            cneg_m = jcb.tile([1, 3], F32, tag='cneg_m', name='cneg_m')
            nc.vector.tensor_scalar(cneg_m[:], cneg[:], msum_ps[:, 0:1], None,
                                    OP.mult)

            c_ps = pmm.tile([3, 3], F32, tag='mm_ps', name='mm_ps', space='PSUM')
            for c in range(4):
                mm(c_ps[:], x_tm[:, 3 * c:3 * c + 3], x_tm[:, 3 * c:3 * c + 3],
                   c == 0, False)
            mm(c_ps[:], cneg[:], mx_sb[:], False, False)
            mm(c_ps[:], mx_sb[:], cneg[:], False, False)
            mm(c_ps[:], cneg[:], cneg_m[:], False, True)

            # Jacobi on flat layouts: a6 = [d0,d1,d2,o01,o02,o12], w9 = V^T rows
            c_sb = jcb.tile([3, 3], F32, tag='c_sb', name='c_sb')
            nc.scalar.copy(c_sb[:], c_ps[:])
            crow = []
            for r in range(3):
                row_ps = pmm.tile([1, 3], F32, tag='mm_ps', name='mm_ps',
                                  space='PSUM')
                mm(row_ps[:], ident[0:3, r:r + 1], c_sb[:], True, True)
                cr = jcb.tile([1, 3], F32, tag=f'crow{r}', name=f'crow{r}')
                nc.scalar.copy(cr[:], row_ps[:])
                crow.append(cr)
            a6 = jcb.tile([1, 6], F32, tag='a6', name='a6')
            for (k, (r_, c_)) in enumerate([(0, 0), (1, 1), (2, 2), (0, 1), (0, 2),
                                            (1, 2)]):
                nc.scalar.copy(a6[:, k:k + 1], crow[r_][:, c_:c_ + 1])
            w9 = jcb.tile([1, 9], F32, tag='w9', name='w9')
            nc.vector.memset(w9[:], 0.0)
            for i in range(3):
                nc.vector.memset(w9[:, 4 * i:4 * i + 1], 1.0)

            OIDX = {(0, 1): 3, (0, 2): 4, (1, 2): 5}
            V = nc.vector

            def j1(name):
                return jcb.tile([1, 1], F32, tag=f'j1_{name}', name=f'j1_{name}')

            def j3(name):
                return jcb.tile([1, 3], F32, tag=f'j3_{name}', name=f'j3_{name}')

            for _s in range(SWEEPS):
                for (p_, q_) in [(0, 1), (0, 2), (1, 2)]:
                    apq = a6[:, OIDX[(p_, q_)]:OIDX[(p_, q_)] + 1]
                    dp = a6[:, p_:p_ + 1]
                    dq = a6[:, q_:q_ + 1]
                    half = j1('half')
                    V.tensor_tensor(out=half[:], in0=dq, in1=dp, op=OP.subtract)
                    hsc = j1('hsc')
                    V.tensor_scalar(hsc[:], half[:], 0.5, None, OP.mult)
                    rapq0 = j1('rapq0')
                    V.reciprocal(rapq0[:], apq)
                    rapq = j1('rapq')
                    V.tensor_scalar(rapq[:], rapq0[:], -1e30, 1e30, OP.max, OP.min)
                    th0 = j1('th0')
                    V.tensor_tensor(out=th0[:], in0=hsc[:], in1=rapq[:], op=OP.mult)
                    th = j1('th')
                    V.tensor_scalar(th[:], th0[:], -1e17, 1e17, OP.max, OP.min)
                    th2 = j1('th2')
                    V.tensor_tensor(out=th2[:], in0=th[:], in1=th[:], op=OP.mult)
                    rt = j1('rt')
                    nc.scalar.activation(rt[:], th2[:], AF.Sqrt, bias=1.0)
                    thneg = j1('thneg')
                    V.tensor_scalar(thneg[:], th[:], -1.0, None, OP.mult)
                    absth = j1('absth')
                    V.tensor_tensor(out=absth[:], in0=th[:], in1=thneg[:], op=OP.max)
                    den = j1('den')
                    V.tensor_tensor(out=den[:], in0=absth[:], in1=rt[:], op=OP.add)
                    ge = j1('ge')
                    V.tensor_scalar(ge[:], th[:], 0.0, None, OP.is_ge)
                    sgn = j1('sgn')
                    V.tensor_scalar(sgn[:], ge[:], 2.0, -1.0, OP.mult, OP.add)
                    rden = j1('rden')
                    V.reciprocal(rden[:], den[:])
                    t_ = j1('t_')
                    V.tensor_tensor(out=t_[:], in0=sgn[:], in1=rden[:], op=OP.mult)
                    t2 = j1('t2')
                    V.tensor_tensor(out=t2[:], in0=t_[:], in1=t_[:], op=OP.mult)
                    rt2 = j1('rt2')
                    nc.scalar.activation(rt2[:], t2[:], AF.Sqrt, bias=1.0)
                    cc = j1('cc')
                    V.reciprocal(cc[:], rt2[:])
                    ss = j1('ss')
                    V.tensor_tensor(out=ss[:], in0=t_[:], in1=cc[:], op=OP.mult)
                    tneg = j1('tneg')
                    V.tensor_scalar(tneg[:], t_[:], -1.0, None, OP.mult)
                    ssneg = j1('ssneg')
                    V.tensor_scalar(ssneg[:], ss[:], -1.0, None, OP.mult)
                    V.scalar_tensor_tensor(out=dp, in0=apq, scalar=tneg[:, 0:1],
                                           in1=dp, op0=OP.mult, op1=OP.add)
                    V.scalar_tensor_tensor(out=dq, in0=apq, scalar=t_[:, 0:1],
                                           in1=dq, op0=OP.mult, op1=OP.add)
                    V.memset(apq, 0.0)
                    r_ = 3 - p_ - q_
                    kp = OIDX[(min(p_, r_), max(p_, r_))]
                    kq = OIDX[(min(q_, r_), max(q_, r_))]
                    apr = a6[:, kp:kp + 1]
                    aqr = a6[:, kq:kq + 1]
                    x1 = j1('x1')
                    V.tensor_scalar(x1[:], apr, cc[:, 0:1], None, OP.mult)
                    x2 = j1('x2')
                    V.tensor_scalar(x2[:], apr, ss[:, 0:1], None, OP.mult)
                    V.scalar_tensor_tensor(out=apr, in0=aqr, scalar=ssneg[:, 0:1],
                                           in1=x1[:], op0=OP.mult, op1=OP.add)
                    V.scalar_tensor_tensor(out=aqr, in0=aqr, scalar=cc[:, 0:1],
                                           in1=x2[:], op0=OP.mult, op1=OP.add)
                    wp = w9[:, 3 * p_:3 * p_ + 3]
                    wqr = w9[:, 3 * q_:3 * q_ + 3]
                    y1 = j3('y1')
                    V.tensor_scalar(y1[:], wp, cc[:, 0:1], None, OP.mult)
                    y2 = j3('y2')
                    V.tensor_scalar(y2[:], wp, ss[:, 0:1], None, OP.mult)
                    V.scalar_tensor_tensor(out=wp, in0=wqr, scalar=ssneg[:, 0:1],
                                           in1=y1[:], op0=OP.mult, op1=OP.add)
                    V.scalar_tensor_tensor(out=wqr, in0=wqr, scalar=cc[:, 0:1],
                                           in1=y2[:], op0=OP.mult, op1=OP.add)

            # ascending eigenvalue sort (3-element network)
            for (ai, bi_) in [(0, 1), (0, 2), (1, 2)]:
                da = a6[:, ai:ai + 1]
                db = a6[:, bi_:bi_ + 1]
                cmp = jcb.tile([1, 1], I32, tag='j1_cmp', name='j1_cmp')
                V.tensor_tensor(out=cmp[:], in0=da, in1=db, op=OP.is_le)
                dlo = j1('dlo')
                V.tensor_tensor(out=dlo[:], in0=da, in1=db, op=OP.min)
                dhi = j1('dhi')
                V.tensor_tensor(out=dhi[:], in0=da, in1=db, op=OP.max)
                V.tensor_copy(da, dlo[:])
                V.tensor_copy(db, dhi[:])
                wa = w9[:, 3 * ai:3 * ai + 3]
                wb = w9[:, 3 * bi_:3 * bi_ + 3]
                wlo = j3('wlo')
                V.select(out=wlo[:], mask=cmp[:].to_broadcast([1, 3]),
                         on_true=wa, on_false=wb)
                whi = j3('whi')
                V.select(out=whi[:], mask=cmp[:].to_broadcast([1, 3]),
                         on_true=wb, on_false=wa)
                V.tensor_copy(wa, wlo[:])
                V.tensor_copy(wb, whi[:])

            # spread w9 (1,9) across partitions -> (3,3) via DRAM roundtrip
            nc.sync.dma_start(jac_scr[0:9].rearrange('(a n) -> a n', a=1), w9[:])
            vt_f = jcb.tile([3, 3], F32, tag='vt_f', name='vt_f')
            nc.sync.dma_start(vt_f[:], jac_scr[0:9].rearrange('(r c) -> r c', r=3))
            vt_sb = geo.tile([3, 3], F32, tag='vt_sb', name='vt_sb')
            nc.scalar.copy(vt_sb[:], vt_f[:])

            # F_all (3, 24), Xc_fm (3, N), hX for all frames -> SBUF (fp16)
            f_ps = pmm.tile([3, 24], F32, tag='mm_ps', name='mm_ps', space='PSUM')
            mm(f_ps[:], vt_sb[:], s_sb[:], True, True)
            fa_sb = geo.tile([3, 24], F32, tag='fa_sb', name='fa_sb')
            nc.scalar.copy(fa_sb[:], f_ps[:])

            xc_ps = pau.tile([3, N], F32, tag='au_ps', name='au_ps', space='PSUM')
            mm(xc_ps[:], ident[0:3, 0:3], x_fm[:], True, False)
            mm(xc_ps[:], cneg[:], mask_row[:], False, True)
            xc_fm = geo.tile([3, N], F32, tag='xc_fm', name='xc_fm')
            nc.scalar.copy(xc_fm[:], xc_ps[:])

            hx_all = geo.tile([3, NF * N], F16, tag='hx_all', name='hx_all')
            for o in range(NF):
                hx_ps = pmm.tile([3, N], F32, tag='mm_ps', name='mm_ps',
                                 space='PSUM')
                mm(hx_ps[:], fa_sb[:, 3 * o:3 * o + 3], xc_fm[:], True, True)
                copy_on(eng('hx', o), hx_all[:, o * N:(o + 1) * N], hx_ps[:])

            # ---------------- layers ----------------

            # u tiles, batched per chunk: per lane u0 raw, u1h/u2h (pre-scaled
            # +bias), u3 raw. d=1 blocks are written frame-reversed. Tags cycle
            # (bufs=2) so blk k+2 reuses blk k's buffer.
            u_t = {}

            for lam in range(2):
                # frame-invariant part of layer-0 q
                if lam == 0:
                    qe_ps = pmm.tile([P, N], F32, tag='mm_ps', name='mm_ps',
                                     space='PSUM')
                    mm(qe_ps[:], wq_sb[0][1][:], embT[0][:], True, False)
                    mm(qe_ps[:], wq_sb[0][2][:], embT[1][:], False, True)
                    q_emb = geo.tile([P, N], F16, tag='q_emb', name='q_emb')
                    copy_on(eng('embq'), q_emb[:], qe_ps[:])

                def attention(f):
                    q_ps = pmm.tile([P, N], F32, tag='mm_ps', name='mm_ps',
                                    space='PSUM')
                    if lam == 0:
                        mm(q_ps[:], wq_sb[0][0][:], hx_all[:, f * N:(f + 1) * N],
                           True, False)
                        mm(q_ps[:], ident_h[:], q_emb[:], False, True)
                    else:
                        mm(q_ps[:], wq_sb[1][0][:], h2[(0, 0)][:, f * N:(f + 1) * N],
                           True, False)
                        mm(q_ps[:], wq_sb[1][1][:], h2[(0, 1)][:, f * N:(f + 1) * N],
                           False, True)
                    q_sb = att.tile([P, N], F16, tag='q_sb', name='q_sb')
                    copy_on(eng('q'), q_sb[:], q_ps[:])

                    k_ps = pmm.tile([P, N], F32, tag='mm_ps', name='mm_ps',
                                    space='PSUM')
                    mm(k_ps[:], wk_sb[lam][:], q_sb[:], True, True)
                    k_sb = att.tile([P, N], F16, tag='k_sb', name='k_sb')
                    copy_on(eng('k'), k_sb[:], k_ps[:])

                    q_tm = att.tile([P, N], F16, tag='q_tm', name='q_tm')
                    tpr = pmm.tile([P, N], F16, tag='mm_ps', name='mm_ps',
                                   space='PSUM')
                    for m in range(4):
                        nc.tensor.transpose(tpr[:, m * P:(m + 1) * P],
                                            q_sb[:, m * P:(m + 1) * P],
                                            ident_h[:])
                    copy_on(eng('qtm'), q_tm[:], tpr[:])

                    z_ps = pzt.tile([1, N], F32, tag='zt_ps', name='zt_ps',
                                    space='PSUM')
                    t_ps = pzt.tile([P, N], F32, tag='zt_ps', name='zt_ps',
                                    space='PSUM')
                    for m in range(4):
                        s_ps = psc.tile([P, N], F32, tag='s_ps', name='s_ps',
                                        space='PSUM')
                        mm(s_ps[:], k_sb[:, m * P:(m + 1) * P], q_sb[:], True, True)
                        e_sb = att.tile([P, N], F16, tag='e_sb', name='e_sb')
                        nc.scalar.activation(e_sb[:], s_ps[:], AF.Exp, scale=SCALE)
                        mm(z_ps[:], ones_col[:], e_sb[:], m == 0, m == 3)
                        mm(t_ps[:], q_tm[:, m * P:(m + 1) * P], e_sb[:], m == 0,
                           m == 3)
                    recip = att.tile([1, N], F16, tag='recip', name='recip')
                    nc.vector.reciprocal(recip[:], z_ps[:])
                    rb_ps = pmm.tile([P, N], F32, tag='mm_ps', name='mm_ps',
                                     space='PSUM')
                    mm(rb_ps[:], ones_row[:], recip[:], True, True)
                    rb_sb = att.tile([P, N], F16, tag='rb_sb', name='rb_sb')
                    copy_on(eng('rb'), rb_sb[:], rb_ps[:])
                    tn_sb = att.tile([P, N], F16, tag='tn_sb', name='tn_sb')
                    eng('tn').tensor_tensor(out=tn_sb[:], in0=t_ps[:], in1=rb_sb[:],
                                            op=OP.mult)

                    a_ps = pau.tile([P, N], F32, tag='au_ps', name='au_ps',
                                    space='PSUM')
                    mm(a_ps[:], wv_sb[lam][:], tn_sb[:], True, False)
                    mm(a_ps[:], ident_h[:], q_sb[:], False, True)
                    a_sb = att.tile([P, N], F16, tag='a_sb', name='a_sb')
                    copy_on(eng('acp'), a_sb[:], a_ps[:])

                    # U matmuls -> batched per-lane tiles (d=1 frame-reversed)
                    for j in range(8):
                        u_ps = pau.tile([P, N], F32, tag='au_ps', name='au_ps',
                                        space='PSUM')
                        mm(u_ps[:], wu_sb[lam][:, j * P:(j + 1) * P], a_sb[:],
                           True, True)
                        d = j // 4
                        mt = j % 4
                        fo = f % FBLK
                        dst = u_t[(d, f'u{mt}', f // FBLK)][
                            :, fo * N:(fo + 1) * N]
                        if d == 1:
                            dst = dst[:, ::-1]
                        if mt in (1, 2):
                            e = eng(f'u{mt}')
                            bias = bfh(lam, d) if mt == 1 else brh(lam, d)
                            if e is nc.scalar:
                                e.activation(dst, u_ps[:], AF.Identity,
                                             bias=bias, scale=0.5)
                            else:
                                e.tensor_scalar(dst, u_ps[:], 0.5, bias,
                                                OP.mult, OP.add)
                        elif mt == 0:
                            # u0h = 0.5*u0 (x = (t-1)*u0h later)
                            e = eng('u0')
                            if e is nc.scalar:
                                e.activation(dst, u_ps[:], AF.Identity,
                                             scale=0.5)
                            else:
                                e.tensor_scalar(dst, u_ps[:], 0.5, None,
                                                OP.mult)
                        else:
                            copy_on(eng('u3'), dst, u_ps[:])

                def scan_group(blks):
                    """All direction lanes of a group of FBLK-frame chunks,
                    emitted op-interleaved so the serial chains overlap on each
                    in-order engine stream. Chunk starts are frame starts, so
                    each chunk scans independently (carry resets at frame
                    boundaries via the f-boundary memsets)."""
                    DS = [(d, b) for b in blks for d in (0, 1)]
                    u0 = {d: u_t[(d[0], 'u0', d[1])][:] for d in DS}
                    u1h = {d: u_t[(d[0], 'u1', d[1])][:] for d in DS}
                    u2h = {d: u_t[(d[0], 'u2', d[1])][:] for d in DS}
                    u3 = {d: u_t[(d[0], 'u3', d[1])][:] for d in DS}

                    def st(base, d):
                        return scn.tile([P, CW], F16, tag=f'{base}{d[0]}_{d[1] % 2}',
                                        name=f'{base}{d[0]}')

                    c_buf = {}
                    for d in DS:
                        c_buf[d] = sc2.tile([P, CW + 1], F16,
                                            tag=f'c_buf{d[0]}_{d[1] % 2}',
                                            name=f'c_buf{d[0]}')
                        nc.vector.memset(c_buf[d][:, 0:1], 0.0)

                    # pass 0: gates from c=0; t = tanh(u1h) (u1h = .5*u1+.5bf)
                    t0, f_t, x_t = {}, {}, {}
                    for d in DS:
                        t0[d] = st('t_t', d)
                        nc.scalar.activation(t0[d][:], u1h[d], AF.Tanh)
                    for d in DS:
                        f_t[d] = st('f_t', d)
                        eng('f').tensor_scalar(f_t[d][:], t0[d][:], 0.5, 0.5,
                                               OP.mult, OP.add)
                        nc.vector.memset(f_t[d][:, 0:CW:N], 0.0)
                        x_t[d] = st('xg_t', d)
                        eng('x').scalar_tensor_tensor(
                            out=x_t[d][:], in0=t0[d][:], scalar=1.0, in1=u0[d],
                            op0=OP.subtract, op1=OP.mult)
                    for d in DS:
                        nc.vector.tensor_tensor_scan(
                            out=c_buf[d][:, 1:CW + 1], data0=f_t[d][:],
                            data1=x_t[d][:], initial=0.0, op0=OP.mult,
                            op1=OP.subtract)

                    # pass 1 (final): gates from c_prev. Pass-0 carry values at
                    # interior frame-start slots are dead after this point, so
                    # zero them in place and use one stt.
                    ag, t1, f1, x1 = {}, {}, {}, {}
                    for d in DS:
                        nc.vector.memset(c_buf[d][:, N:CW:N], 0.0)
                        ag[d] = st('xg_t', d)
                        eng('ag').scalar_tensor_tensor(
                            out=ag[d][:], in0=c_buf[d][:, 0:CW],
                            scalar=vfh(lam, d[0]), in1=u1h[d], op0=OP.mult,
                            op1=OP.add)
                    for d in DS:
                        t1[d] = st('t_t', d)
                        nc.scalar.activation(t1[d][:], ag[d][:], AF.Tanh)
                    for d in DS:
                        f1[d] = st('f_t', d)
                        eng('f').tensor_scalar(f1[d][:], t1[d][:], 0.5, 0.5,
                                               OP.mult, OP.add)
                        nc.vector.memset(f1[d][:, 0:CW:N], 0.0)
                        x1[d] = st('xg_t', d)
                        eng('x').scalar_tensor_tensor(
                            out=x1[d][:], in0=t1[d][:], scalar=1.0, in1=u0[d],
                            op0=OP.subtract, op1=OP.mult)
                    for d in DS:
                        nc.vector.tensor_tensor_scan(
                            out=c_buf[d][:, 1:CW + 1], data0=f1[d][:],
                            data1=x1[d][:], initial=0.0, op0=OP.mult,
                            op1=OP.subtract)

                    # r gate + output. D (which needs the true carry at frame
                    # ends) is computed BEFORE zeroing the frame-start slots.
                    ar, tr, D_t, r_t, m_t = {}, {}, {}, {}, {}
                    for d in DS:
                        D_t[d] = st('fm_t', d)
                        eng('D').tensor_tensor(out=D_t[d][:],
                                               in0=c_buf[d][:, 1:CW + 1],
                                               in1=u3[d], op=OP.subtract)
                    for d in DS:
                        nc.vector.memset(c_buf[d][:, N:CW:N], 0.0)
                        ar[d] = st('xg_t', d)
                        eng('ar').scalar_tensor_tensor(
                            out=ar[d][:], in0=c_buf[d][:, 0:CW],
                            scalar=vrh(lam, d[0]), in1=u2h[d], op0=OP.mult,
                            op1=OP.add)
                    for d in DS:
                        tr[d] = st('t_t', d)
                        nc.scalar.activation(tr[d][:], ar[d][:], AF.Tanh)
                        r_t[d] = st('f_t', d)
                        eng('r').tensor_scalar(r_t[d][:], tr[d][:], 0.5, 0.5,
                                               OP.mult, OP.add)
                        m_t[d] = st('xg_t', d)
                        eng('m').tensor_tensor(out=m_t[d][:], in0=r_t[d][:],
                                               in1=D_t[d][:], op=OP.mult)
                    for d in DS:
                        blk = d[1]
                        c0 = blk * CW
                        dst = h2[(lam, d[0])]
                        if d[0] == 0:
                            eng('hf').tensor_tensor(out=dst[:, c0:c0 + CW],
                                                    in0=m_t[d][:], in1=u3[d],
                                                    op=OP.add)
                        else:
                            dv = dst.rearrange('p (f l) -> p f l', f=NF)[
                                :, blk * FBLK:(blk + 1) * FBLK, ::-1]
                            eng('hf').tensor_tensor(
                                out=dv,
                                in0=m_t[d][:].rearrange('p (f l) -> p f l',
                                                        f=FBLK),
                                in1=u3[d].rearrange('p (f l) -> p f l', f=FBLK),
                                op=OP.add)

                GRP = 2  # chunks per scan group (4 chains in flight)
                for g in range(0, NBLK, GRP):
                    blks = list(range(g, min(g + GRP, NBLK)))
                    for blk in blks:
                        for d in range(2):
                            for nm in ('u0', 'u1', 'u2', 'u3'):
                                u_t[(d, nm, blk)] = ust.tile(
                                    [P, CW], F16, tag=f'{nm}_{d}_{blk % 2}',
                                    name=f'{nm}_{d}')
                        for f in range(blk * FBLK, (blk + 1) * FBLK):
                            attention(f)
                    scan_group(blks)

        def out_section():
            """Mean over frames + transpose + store for the most recent h2.
            Emitted at the TOP of the loop body (software pipelining), reading
            the previous runtime iteration's h2, so each iteration's tail ends
            at the last scan and PE/ACT get ready work at iteration start."""
            h2d, ident = ST['h2'], ST['ident']
            for t in range(2):
                hv = h2d[(1, t)]
                s1 = mnp.tile([P, 4 * N], F16, tag=f'ms1_{t}', name=f'ms1_{t}')
                eng('mean').tensor_tensor(out=s1[:], in0=hv[:, 0:4 * N],
                                          in1=hv[:, 4 * N:8 * N], op=OP.add)
                s2 = mnp.tile([P, 2 * N], F16, tag=f'ms2_{t}', name=f'ms2_{t}')
                eng('mean').tensor_tensor(out=s2[:], in0=s1[:, 0:2 * N],
                                          in1=s1[:, 2 * N:4 * N], op=OP.add)
                m_fm = mnp.tile([P, N], F32, tag=f'm_fm{t}', name=f'm_fm{t}')
                eng('mean').tensor_tensor(out=m_fm[:], in0=s2[:, 0:N],
                                          in1=s2[:, N:2 * N], op=OP.add)
                for c in range(4):
                    tp = pmm.tile([P, P], F32, tag='mm_ps', name='mm_ps',
                                  space='PSUM')
                    nc.tensor.transpose(tp[:], m_fm[:, c * P:(c + 1) * P],
                                        ident[:])
                    o_st = pre.tile([P, P], F32, tag='o_st', name='o_st')
                    nc.scalar.activation(o_st[:], tp[:], AF.Copy, scale=1.0 / NF)
                    nc.sync.dma_start(out_d[c * P:(c + 1) * P,
                                            t * P:(t + 1) * P], o_st[:])

        with nc.allow_low_precision(reason='fp16 pipeline, within 2e-2 tol'):
            ST = init_static()
            if n_iter == 1:
                body()
            elif UNROLL:
                for i in range(n_iter):
                    if i > 0:
                        out_section()
                    body()
            else:
                with tc.For_i(0, n_iter, 1):
                    out_section()
                    body()
            out_section()

    nc.compile()
    return nc


_CACHE = {}


def _get_nc(n_iter=1):
    if n_iter not in _CACHE:
        _CACHE[n_iter] = _build(n_iter)
    return _CACHE[n_iter]


def host_inputs(inputs):
    """Build the 8 per-core input maps (pure slicing/packing/indexing)."""
    ops = np.array([[i, j, k] for i in (-1, 1) for j in (-1, 1) for k in (-1, 1)],
                   np.float32)
    S = np.zeros((3, 24), np.float32)
    for o in range(8):
        S[:, 3 * o:3 * o + 3] = np.diag(ops[o])

    shared = {'sops': S}
    for lam in range(2):
        shared[f'wq{lam}'] = np.ascontiguousarray(inputs[f'Wq{lam}'], np.float16)
        shared[f'wk{lam}'] = np.ascontiguousarray(inputs[f'Wk{lam}'], np.float16)
        shared[f'wv{lam}'] = np.ascontiguousarray(inputs[f'Wv{lam}'], np.float16)
        shared[f'wu{lam}'] = np.ascontiguousarray(inputs[f'Wu{lam}'], np.float16)
        shared[f'gates{lam}'] = np.ascontiguousarray(np.concatenate(
            [inputs[f'vf{lam}'], inputs[f'vr{lam}'],
             inputs[f'bf{lam}'], inputs[f'br{lam}']]), np.float32)

    emb = np.asarray(inputs['emb'], np.float32)
    seqs = np.asarray(inputs['noisy_seqs'])
    seqs = np.where(seqs < 0, 82, seqs)

    in_maps = []
    for b in range(B):
        m = dict(shared)
        m['x_ca'] = np.ascontiguousarray(inputs['noisy_cords'][b, :, 1, :],
                                         np.float32)
        m['maskr'] = np.ascontiguousarray(
            np.asarray(inputs['mask'][b], np.float32).reshape(1, N))
        m['embt'] = np.ascontiguousarray(emb[seqs[b]].T, np.float16)
        in_maps.append(m)
    return in_maps


def kernel(**inputs):
    nc = _get_nc(1)
    in_maps = host_inputs(inputs)
    res = run_bass_kernel_spmd(nc, in_maps, core_ids=list(range(B)))
    return np.stack([res.results[b]['out'] for b in range(B)], axis=0)
